# revision 1
# baseline (speedup 1.0000x reference)
"""GCBlock GNN message-passing kernel for 8 Trainium2 NeuronCores.

Strategy:
  * Host: sort edges by destination idx_i, shard at node boundaries into 8
    balanced slices (each core owns a disjoint output node range -> no
    collectives), pack edges into 128-edge tiles that never split a node,
    fold pi_w2 @ ii_w1 into a single W_mid (no nonlinearity between them).
  * Device phase A: every core computes the full pp1 = MLP(p1) node table
    into a DRAM scratch (feature-major matmuls, tanh on ScalarE).
  * Device phase B (per 512-edge chunk = 4 tiles): per-tile indirect-DMA
    gathers of pp1 rows for idx_i/idx_j (one index per partition -- the only
    pattern the SWDGE ucode supports), DVE add, PE transposes into PSUM, add
    host-pre-transposed basis, 3 matmul layers (bf16, fp32 PSUM), tanh on
    ScalarE, one-hot scatter matmuls into a 32-node window PSUM, then ONE
    static HWDGE write of the 4 windows to a DRAM staging buffer (each node
    lives in exactly one tile -> windows are disjoint).
  * Device phase C: compact staging rows to output rows with ~1 indirect
    gather per 128 output rows (host-computed map). This keeps the SWDGE
    instruction count low -- serialized indirect-DMA issue (~1.4us each) is
    the dominant cost on this workload, not bytes or FLOPs. The remaining
    indirect DMAs are spread round-robin over two SWDGE queues
    (num_swdge_queues=2), which roughly halves their serialized cost.
"""

import math

import numpy as np

import concourse.bacc as bacc
import concourse.bass as bass
import concourse.mybir as mybir
from concourse.bass import IndirectOffsetOnAxis
from concourse.bass_utils import run_bass_kernel_spmd
from concourse.tile import TileContext

D = 64
TILE = 128          # edges per tile
TPC = 4             # tiles per chunk
CHUNK = TILE * TPC  # 512 edges/nodes per chunk
WIN = 32            # scatter window rows per tile
NCORES = 8
PAD_LOC = 300.0     # one-hot local index for pad edges (matches nothing)

SWDGE_QUEUES = 2


def make_nc():
    return bacc.Bacc(trn_type="TRN2", num_swdge_queues=SWDGE_QUEUES)


FP = mybir.dt.float32
FR = mybir.dt.float32r
NPF = np.float32

USE_BF16 = True
BF = mybir.dt.bfloat16
if USE_BF16:
    import ml_dtypes
    NPB = ml_dtypes.bfloat16
    DT = BF
    NPD = NPB
else:
    DT = FP
    NPD = NPF

# tensors that move to bf16 when USE_BF16 (host side)
BF_CONSTS = ["p1t", "w1pp", "w2pp", "w1pi", "wmid", "w2ii", "ident", "iota",
             "ones_row", "bpp2_row", "bii2_row"]
BF_PER_CORE = ["basis_p", "locf"]


def _table_row(g):
    """Physical row of node g in the packed pp1 table ([rows, 64] view)."""
    return (g // CHUNK) * 512 + (g % 128) * 4 + (g % CHUNK) // 128


# ---------------------------------------------------------------- host prep

def _pack_fm(tiles_em):
    """tiles_em: [4, 128, 64] edge-major tiles -> [64, 512] FM."""
    out = np.zeros((64, 512), dtype=NPF)
    for k in range(TPC):
        out[:, 128 * k:128 * k + 128] = tiles_em[k].T
    return out


def prep(idx_i, idx_j, p1, basis, weights):
    N, E = p1.shape[0], idx_i.shape[0]
    NA = math.ceil(N / CHUNK)

    order = np.argsort(idx_i, kind="stable")
    si = idx_i[order]
    sj = idx_j[order]
    sb = basis[order]

    # core boundaries snapped to node edges, balancing edge counts
    node_bounds = [0]
    edge_bounds = [0]
    for c in range(1, NCORES):
        pos = min(int(round(c * E / NCORES)), E - 1)
        node_c = max(int(si[pos]), node_bounds[-1] + 1)
        node_bounds.append(node_c)
        edge_bounds.append(int(np.searchsorted(si, node_c)))
    node_bounds.append(N)
    edge_bounds.append(E)

    # per-core tile packing (no node spans two tiles; window spread < WIN)
    core_tiles = []
    for c in range(NCORES):
        s, e = edge_bounds[c], edge_bounds[c + 1]
        nb = node_bounds[c]
        loc_nodes = si[s:e] - nb
        nsl = node_bounds[c + 1] - nb
        deg = np.bincount(loc_nodes, minlength=nsl)
        nz = np.flatnonzero(deg)
        node_estart = s + np.concatenate([[0], np.cumsum(deg)[:-1]])
        firsts, lasts, estarts, ecounts = [], [], [], []
        cur_first = None
        for n in nz:
            d = int(deg[n])
            assert d <= TILE, f"node degree {d} > {TILE} unsupported"
            if cur_first is None or cur_cnt + d > TILE or n - cur_first >= WIN:
                if cur_first is not None:
                    firsts.append(cur_first)
                    lasts.append(cur_last)
                    estarts.append(cur_es)
                    ecounts.append(cur_cnt)
                cur_first, cur_cnt, cur_es = int(n), 0, int(node_estart[n])
            cur_cnt += d
            cur_last = int(n)
        if cur_first is not None:
            firsts.append(cur_first)
            lasts.append(cur_last)
            estarts.append(cur_es)
            ecounts.append(cur_cnt)
        core_tiles.append((firsts, lasts, estarts, ecounts))

    NT = max(len(t[0]) for t in core_tiles)
    NCHUNK = math.ceil(NT / TPC)
    NT = NCHUNK * TPC
    NSL = max(node_bounds[c + 1] - node_bounds[c] for c in range(NCORES))
    DUMP = NSL
    NBLKF = math.ceil((NSL + 1) / 128)

    per_core = []
    for c in range(NCORES):
        firsts, lasts, estarts, ecounts = core_tiles[c]
        nb = node_bounds[c]
        basis_p = np.zeros((NCHUNK, 64, 512), dtype=NPF)
        gidx = np.zeros((NCHUNK, 128, TPC), dtype=np.int32)
        gjdx = np.zeros((NCHUNK, 128, TPC), dtype=np.int32)
        locf = np.full((NCHUNK, 128, TPC), PAD_LOC, dtype=NPF)
        scat = np.full((NCHUNK, WIN, TPC), DUMP, dtype=np.int32)
        tiles_em = np.zeros((TPC, 128, D), dtype=NPF)
        for ch in range(NCHUNK):
            tiles_em[:] = 0.0
            for k in range(TPC):
                t = ch * TPC + k
                if t >= len(firsts):
                    continue
                es, cnt, fn, ln = estarts[t], ecounts[t], firsts[t], lasts[t]
                tiles_em[k, :cnt] = sb[es:es + cnt]
                gidx[ch, :cnt, k] = si[es:es + cnt]
                gjdx[ch, :cnt, k] = sj[es:es + cnt]
                locf[ch, :cnt, k] = (si[es:es + cnt] - nb - fn).astype(NPF)
                nrows = ln - fn + 1
                scat[ch, :nrows, k] = np.arange(fn, ln + 1)
            basis_p[ch] = _pack_fm(tiles_em)
        gidx = _table_row(gidx.astype(np.int64)).astype(np.int32)
        gjdx = _table_row(gjdx.astype(np.int64)).astype(np.int32)
        # final-pass compaction: output row n <- stage row 32*t + (n - first_t)
        fidx = np.zeros((NBLKF * 128,), dtype=np.int32)
        for t in range(len(firsts)):
            fn, ln = firsts[t], lasts[t]
            fidx[fn:ln + 1] = t * WIN + np.arange(ln + 1 - fn)
        fidx = fidx.reshape(NBLKF, 128, 1)
        per_core.append(dict(basis_p=basis_p, gidx=gidx, gjdx=gjdx,
                             locf=locf, scat=scat, fidx=fidx))

    # phase A packing (same for all cores)
    p1_pad = np.zeros((NA * CHUNK, D), dtype=NPF)
    p1_pad[:N] = p1
    p1t = np.zeros((NA, 64, 512), dtype=NPF)
    for a in range(NA):
        p1t[a] = _pack_fm(p1_pad[a * CHUNK:(a + 1) * CHUNK].reshape(TPC, 128, D))

    w = weights
    W_mid = (w["pi_w2"] @ w["ii_w1"]).astype(NPF)
    b_mid = (w["pi_b2"] @ w["ii_w1"] + w["ii_b1"]).astype(NPF)

    consts = dict(
        p1t=p1t,
        w1pp=w["pp_w1"].astype(NPF), w2pp=w["pp_w2"].astype(NPF),
        w1pi=w["pi_w1"].astype(NPF), wmid=W_mid,
        w2ii=w["ii_w2"].astype(NPF),
        ident=np.eye(128, dtype=NPF),
        iota=np.tile(np.arange(WIN, dtype=NPF), (128, 1)),
        b_pp1=w["pp_b1"].reshape(64, 1).astype(NPF),
        b_pi1=w["pi_b1"].reshape(64, 1).astype(NPF),
        b_mid=b_mid.reshape(64, 1),
        ones_row=np.ones((1, 128), dtype=NPF),
        bpp2_row=w["pp_b2"].reshape(1, D).astype(NPF),
        bii2_row=w["ii_b2"].reshape(1, D).astype(NPF),
    )
    if USE_BF16:
        for nm in BF_CONSTS:
            consts[nm] = consts[nm].astype(NPB)
        for pc in per_core:
            for nm in BF_PER_CORE:
                pc[nm] = pc[nm].astype(NPB)

    dims = dict(N=N, E=E, NA=NA, NCHUNK=NCHUNK, NSL=NSL, NBLKF=NBLKF,
                node_bounds=node_bounds)
    return per_core, consts, dims


# ------------------------------------------------------------- device build

def build(nc, dims, consts, sections=("A", "B")):
    import os
    _NOGATHER = bool(os.environ.get("GC_NOGATHER"))
    NA, NCHUNK, NSL = dims["NA"], dims["NCHUNK"], dims["NSL"]
    has_bpp2 = bool(np.any(consts["bpp2_row"] != 0))
    has_bii2 = bool(np.any(consts["bii2_row"] != 0))
    has_bpp1 = bool(np.any(consts["b_pp1"] != 0))
    has_bpi1 = bool(np.any(consts["b_pi1"] != 0))
    has_bmid = bool(np.any(consts["b_mid"] != 0))

    t_p1t = nc.dram_tensor("p1t", (NA, 64, 512), DT, kind="ExternalInput")
    t_basis = nc.dram_tensor("basis_p", (NCHUNK, 64, 512), DT, kind="ExternalInput")
    t_gidx = nc.dram_tensor("gidx", (NCHUNK, 128, TPC), mybir.dt.int32, kind="ExternalInput")
    t_gjdx = nc.dram_tensor("gjdx", (NCHUNK, 128, TPC), mybir.dt.int32, kind="ExternalInput")
    t_locf = nc.dram_tensor("locf", (NCHUNK, 128, TPC), DT, kind="ExternalInput")
    t_fidx = nc.dram_tensor("fidx", (dims["NBLKF"], 128, 1), mybir.dt.int32, kind="ExternalInput")
    cts = {}
    cdt = {}
    for nm in ["w1pp", "w2pp", "w1pi", "wmid", "w2ii", "ident", "iota",
               "b_pp1", "b_pi1", "b_mid", "ones_row", "bpp2_row", "bii2_row"]:
        cdt[nm] = DT if (USE_BF16 and nm in BF_CONSTS) else FP
        cts[nm] = nc.dram_tensor(nm, consts[nm].shape, cdt[nm], kind="ExternalInput")
    NBLKF = dims["NBLKF"]
    t_out = nc.dram_tensor("out", (NBLKF * 128, D), FP, kind="ExternalOutput")
    table = nc.dram_tensor("pp1_table", (NA * 128, 256), DT, kind="Internal")
    stage = nc.dram_tensor("stage", (NCHUNK * TPC * WIN, D), FP, kind="Internal")
    table_rows = table[:].rearrange("r (k f) -> (r k) f", k=TPC)  # [NA*512, 64]

    def load_consts(pool):
        sb = {}
        for nm, t in cts.items():
            tile = pool.tile(list(consts[nm].shape), cdt[nm], tag=nm)
            nc.sync.dma_start(tile[:], t[:])
            sb[nm] = tile
        return sb

    Tanh = mybir.ActivationFunctionType.Tanh
    Copy = mybir.ActivationFunctionType.Copy

    def mm(out, lhsT, rhs, start=True, stop=True):
        nc.tensor.matmul(out, lhsT=lhsT, rhs=rhs, start=start, stop=stop)

    # EM layer: psum [128, 256] col-block k <- h[:, 128k:+128].T @ w (+ bias)
    def em_layer(ps, h, w_sb, bias_row, has_bias, sbk):
        for k in range(TPC):
            mm(ps[:, 64 * k:64 * k + 64], h[:, 128 * k:128 * k + 128],
               w_sb[:], start=True, stop=not has_bias)
            if has_bias:
                mm(ps[:, 64 * k:64 * k + 64], sbk["ones_row"][:, :],
                   bias_row[:, :], start=False, stop=True)

    # ---------------- phase A: pp1 table ----------------
    na = NA if "A" in sections else 1
    with TileContext(nc) as tc:
        with tc.tile_pool(name="cst", bufs=1) as cpool, \
             tc.tile_pool(name="sba", bufs=3) as pool, \
             tc.tile_pool(name="psa", bufs=2, space="PSUM") as pspool:
            sbk = load_consts(cpool)
            for a in range(na):
                p1c = pool.tile([64, 512], DT, tag="p1c")
                nc.sync.dma_start(p1c[:], t_p1t[a])
                ps1 = pspool.tile([64, 512], FP, tag="ps1")
                mm(ps1[:], sbk["w1pp"][:], p1c[:])
                h1 = pool.tile([64, 512], DT, tag="h1a")
                if has_bpp1:
                    nc.scalar.activation(h1[:], ps1[:], Tanh, bias=sbk["b_pp1"][:])
                else:
                    nc.scalar.activation(h1[:], ps1[:], Tanh)
                ps2 = pspool.tile([128, 256], FP, tag="ps2")
                em_layer(ps2, h1, sbk["w2pp"], sbk["bpp2_row"], has_bpp2, sbk)
                pe = pool.tile([128, 256], DT, tag="pea")
                nc.vector.tensor_copy(pe[:], ps2[:])
                nc.sync.dma_start(table[a * 128:(a + 1) * 128, :], pe[:])

    # ---------------- phase B: edges ----------------
    nch = NCHUNK if "B" in sections else 0
    with TileContext(nc) as tc:
        with tc.tile_pool(name="cstb", bufs=1) as cpool, \
             tc.tile_pool(name="sbb", bufs=4) as pool, \
             tc.tile_pool(name="meta", bufs=4) as mpool, \
             tc.tile_pool(name="psI", bufs=2, space="PSUM") as psI, \
             tc.tile_pool(name="psH", bufs=1, space="PSUM") as psH, \
             tc.tile_pool(name="psE", bufs=1, space="PSUM") as psE, \
             tc.tile_pool(name="psS", bufs=2, space="PSUM") as psS:
            sbk = load_consts(cpool)
            for ch in range(nch):
                bas = pool.tile([64, 512], DT, tag="bas")
                nc.sync.dma_start(bas[:], t_basis[ch])
                gi_sb = mpool.tile([128, TPC], mybir.dt.int32, tag="gi")
                nc.sync.dma_start(gi_sb[:], t_gidx[ch])
                gj_sb = mpool.tile([128, TPC], mybir.dt.int32, tag="gj")
                nc.sync.dma_start(gj_sb[:], t_gjdx[ch])
                loc_sb = mpool.tile([128, TPC], DT, tag="loc")
                nc.sync.dma_start(loc_sb[:], t_locf[ch])
                graw = pool.tile([128, 256], DT, tag="graw")
                gjraw = pool.tile([128, 256], DT, tag="gjraw")
                if _NOGATHER:
                    r0 = (ch % NA) * 128
                    nc.sync.dma_start(graw[:], table[r0:r0 + 128, :])
                    nc.sync.dma_start(gjraw[:], table[r0:r0 + 128, :])
                else:
                    for k in range(TPC):
                        i1 = nc.gpsimd.indirect_dma_start(
                            out=graw[:, 64 * k:64 * k + 64], out_offset=None,
                            in_=table_rows,
                            in_offset=IndirectOffsetOnAxis(ap=gi_sb[:, k:k + 1], axis=0))
                        i2 = nc.gpsimd.indirect_dma_start(
                            out=gjraw[:, 64 * k:64 * k + 64], out_offset=None,
                            in_=table_rows,
                            in_offset=IndirectOffsetOnAxis(ap=gj_sb[:, k:k + 1], axis=0))
                        i2.ins.queue = "qPoolDynamic1"
                gsum = pool.tile([128, 256], DT, tag="gsum")
                nc.vector.tensor_tensor(out=gsum[:], in0=graw[:], in1=gjraw[:],
                                        op=mybir.AluOpType.add)

                psi = psI.tile([64, 512], DT, tag="psi")
                for k in range(TPC):
                    nc.tensor.matmul(psi[:, 128 * k:128 * k + 128],
                                     lhsT=gsum[:, 64 * k:64 * k + 64],
                                     rhs=sbk["ident"][:], is_transpose=True,
                                     start=True, stop=True)
                interf = pool.tile([64, 512], DT, tag="interf")
                nc.vector.tensor_tensor(out=interf[:], in0=psi[:], in1=bas[:],
                                        op=mybir.AluOpType.add)

                ph1 = psH.tile([64, 512], FP, tag="ph1")
                mm(ph1[:], sbk["w1pi"][:], interf[:])
                h1 = pool.tile([64, 512], DT, tag="h1")
                if has_bpi1:
                    nc.scalar.activation(h1[:], ph1[:], Tanh, bias=sbk["b_pi1"][:])
                else:
                    nc.scalar.activation(h1[:], ph1[:], Tanh)

                ph2 = psH.tile([64, 512], FP, tag="ph2")
                mm(ph2[:], sbk["wmid"][:], h1[:])
                h2 = pool.tile([64, 512], DT, tag="h2")
                if has_bmid:
                    nc.scalar.activation(h2[:], ph2[:], Tanh, bias=sbk["b_mid"][:])
                else:
                    nc.scalar.activation(h2[:], ph2[:], Tanh)

                pse = psE.tile([128, 256], FP, tag="pse")
                em_layer(pse, h2, sbk["w2ii"], sbk["bii2_row"], has_bii2, sbk)
                iiem = pool.tile([128, 256], DT, tag="iiem")
                nc.scalar.activation(iiem[:], pse[:], Copy)

                pss = psS.tile([WIN, 256], FP, tag="pss")
                for k in range(TPC):
                    oh = mpool.tile([128, WIN], DT, tag=f"oh{k % 2}")
                    nc.vector.tensor_tensor(
                        out=oh[:],
                        in0=loc_sb[:, k:k + 1].to_broadcast([128, WIN]),
                        in1=sbk["iota"][:, :],
                        op=mybir.AluOpType.is_equal)
                    mm(pss[:, 64 * k:64 * k + 64], oh[:],
                       iiem[:, 64 * k:64 * k + 64])
                s_sb = pool.tile([WIN, 256], FP, tag="s_sb")
                nc.vector.tensor_copy(s_sb[:], pss[:])
                st = stage[ch * TPC * WIN:(ch + 1) * TPC * WIN, :]
                nc.sync.dma_start(
                    st.rearrange("(k p) f -> p k f", k=TPC),
                    s_sb[:].rearrange("p (k f) -> p k f", k=TPC))
    # ---------------- phase C: compact stage -> out ----------------
    with TileContext(nc) as tc:
        with tc.tile_pool(name="sbc", bufs=4) as pool, \
             tc.tile_pool(name="metac", bufs=4) as mpool:
            for b in range(NBLKF if "B" in sections else 0):
                fx = mpool.tile([128, 1], mybir.dt.int32, tag="fx")
                nc.sync.dma_start(fx[:], t_fidx[b])
                g = pool.tile([128, D], FP, tag="g")
                ic = nc.gpsimd.indirect_dma_start(
                    out=g[:], out_offset=None, in_=stage[:],
                    in_offset=IndirectOffsetOnAxis(ap=fx[:], axis=0))
                if b % 2:
                    ic.ins.queue = "qPoolDynamic1"
                nc.sync.dma_start(t_out[b * 128:(b + 1) * 128, :], g[:])
    nc.compile()


# ----------------------------------------------------------------- kernel()

SHARED_NAMES = ["w1pp", "w2pp", "w1pi", "wmid", "w2ii", "ident", "iota",
                "b_pp1", "b_pi1", "b_mid", "ones_row", "bpp2_row",
                "bii2_row", "p1t"]
PER_CORE_NAMES = ["basis_p", "gidx", "gjdx", "locf", "fidx"]


def make_in_maps(per_core, consts):
    shared = {nm: consts[nm] for nm in SHARED_NAMES}
    in_maps = []
    for c in range(NCORES):
        m = dict(shared)
        for nm in PER_CORE_NAMES:
            m[nm] = per_core[c][nm]
        in_maps.append(m)
    return in_maps


def kernel(**inputs):
    idx_i = np.asarray(inputs["idx_i"]).astype(np.int64)
    idx_j = np.asarray(inputs["idx_j"]).astype(np.int64)
    p1 = np.asarray(inputs["p1"], dtype=NPF)
    basis = np.asarray(inputs["basis"], dtype=NPF)
    weights = {k: np.asarray(inputs[k], dtype=NPF) for k in
               ["pp_w1", "pp_b1", "pp_w2", "pp_b2",
                "pi_w1", "pi_b1", "pi_w2", "pi_b2",
                "ii_w1", "ii_b1", "ii_w2", "ii_b2"]}

    per_core, consts, dims = prep(idx_i, idx_j, p1, basis, weights)

    nc = make_nc()
    build(nc, dims, consts)

    import os
    trace = bool(os.environ.get("GC_TRACE"))
    res = run_bass_kernel_spmd(nc, make_in_maps(per_core, consts),
                               core_ids=list(range(NCORES)), trace=trace)
    global LAST_EXEC_NS
    LAST_EXEC_NS = res.exec_time_ns

    N = dims["N"]
    nbs = dims["node_bounds"]
    out = np.zeros((N, D), dtype=NPF)
    for c in range(NCORES):
        out[nbs[c]:nbs[c + 1]] = res.results[c]["out"][:nbs[c + 1] - nbs[c]]
    deg = np.bincount(idx_i, minlength=N)
    out[deg == 0] = 0
    return out



# revision 17
# speedup vs baseline: 6.8368x; 6.8368x over previous
"""GCBlock GNN message-passing kernel for 8 Trainium2 NeuronCores.

Strategy (v5 — bulk int16 dma_gather, 4096-row batches):
  * Host: shard edges by destination node range (each core owns a disjoint
    output range -> no collectives). Within a core, sort edges by
    (j-block, i) where j-blocks are 25600-node ranges, so that j-gather
    indices are block-local int16 and i-gather indices are core-local int16.
    Pack edges into 128-edge tiles of whole nodes with node span < 96.
  * Device phase A: compute the pp1 = MLP(p1) node table into DRAM twice:
    an i-table holding this core's node range and four j-block tables,
    bf16 rows padded to 256B (the gather stride field is in 256B units).
    Inputs arrive host-packed in stacked-pair FM layout ([128,512] = two
    64-feature panels on the partition axis) so matmuls/tanh run with
    block-diagonal weights at full 128-partition width.
  * Device phase B (per 4-group batch = 4096 edges): ONE 4096-row
    dma_gather for i-rows + ONE for j-rows (994ns fixed SWDGE cost
    amortized over 4096 descriptors; descriptors read only the 128B
    payload of each 256B row). Per 1024-edge group: DVE add, PE transposes
    to stacked FM, DVE basis add, 3 matmul layers with block-diagonal
    weights, tanh on ScalarE, one-hot scatter matmuls into per-tile
    96-row windows, one psum->sbuf copy and ONE static write of all 8
    windows into a tile-major bf16 stage tensor.
  * Device phase C: per j-block, TWO dma_gathers fetch every output row's
    stage partial; 3 DVE adds + one static write produce the final
    segment sums.
  * All data-dependent structure lives in index tensors; the instruction
    schedule is identical across cores (SPMD single program).
"""

import math
import os

import numpy as np
import ml_dtypes

import concourse.bacc as bacc
import concourse.bass as bass
import concourse.mybir as mybir
from concourse.bass_utils import run_bass_kernel_spmd
from concourse.tile import TileContext

D = 64
TILE = 128            # edges per tile
TPG = 8               # tiles per group
GRP = TILE * TPG      # 1024 edges per group
GB = 4                # groups per gather batch (4096 edges)
BATCH = GRP * GB
MWB = BATCH // 16     # idx columns after 16-partition wrap (256)
NCORES = 8
JB = 25600            # j-block size (int16-safe, multiple of 1024)
NJB = 4
PAD_LOC = 300.0       # one-hot local index for pad edges (matches nothing)
AB = 2                # phase-A steps per load batch

FP = mybir.dt.float32
BF = mybir.dt.bfloat16
I16 = mybir.dt.int16
NPF = np.float32
NPB = ml_dtypes.bfloat16


def make_nc():
    return bacc.Bacc(trn_type="TRN2", num_swdge_queues=2)


def dma_gather_raw(nc, out_ap, in_ap, idxs_ap, num_idxs, elem_size,
                   elem_step, queue_num=0):
    """dma_gather without the helper's 256B elem minimum / 1024-idx packet.

    The ISA stride field is in 256B units (stride must be %256), but the
    per-descriptor read size is free — reading the 128B payload of padded
    256B rows halves DMA-engine time vs. gathering the full padded row.
    single_packet=False lets num_idxs exceed the 1024-descriptor ring.
    """
    from concourse import ap_utils
    g = nc.gpsimd
    assert idxs_ap.dtype == I16
    assert in_ap.dtype == out_ap.dtype
    stride_bytes = elem_step * mybir.dt.size(in_ap.dtype)
    stride_bytes_256 = stride_bytes // 256
    assert stride_bytes_256 * 256 == stride_bytes and stride_bytes_256 < 256
    assert ap_utils.ap_is_contiguous(out_ap.ap[1:])
    assert ap_utils.ap_is_contiguous(idxs_ap.ap[1:])
    assert in_ap.ap[0][0] == elem_step
    assert in_ap.ap[-1][1] == elem_size
    assert out_ap.ap[-1][1] == elem_size
    _in_ap = g.lower_ap_dma(in_ap, for_custom_bir_dma=True)
    _idxs_ap = g.lower_ap(idxs_ap)
    _out_ap = g.lower_ap(out_ap)
    return g.add_instruction(
        mybir.InstDMAGatherAnt(
            name=g.bass.get_next_instruction_name(),
            ins=[*_in_ap, _idxs_ap, g.lower_val_access(g.to_reg(num_idxs))],
            outs=[_out_ap],
            transpose=False,
            num_idxs=num_idxs,
            elem_size=elem_size,
            stride_bytes_256=stride_bytes_256,
            gen_mode=0,
            single_packet=False,
            queue_num=queue_num,
            sbuf_tokens_per_rank=0,
            sbuf_free_dim_per_rank=0,
            sbuf_free_dim_pad_per_rank=0,
            sbuf_byte_offset=0,
        ))


def _wrap16(lin):
    """[n] int16 linear index list -> [128, n//16] SWDGE-wrapped+replicated."""
    n = lin.shape[0]
    w = lin.reshape(n // 16, 16).T
    return np.tile(w, (8, 1)).copy()


def _bd(w):
    """64x64 -> 128x128 block-diagonal (stacked-pair weights)."""
    out = np.zeros((128, 128), dtype=w.dtype)
    out[:64, :64] = w
    out[64:, 64:] = w
    return out


# ---------------------------------------------------------------- host prep

def prep(idx_i, idx_j, p1, basis, weights):
    N, E = p1.shape[0], idx_i.shape[0]
    assert N <= NJB * JB

    order = np.argsort(idx_i, kind="stable")
    si_all = idx_i[order]
    sj_all = idx_j[order]
    sb_all = basis[order]

    # core boundaries snapped to node edges, balancing edge counts
    node_bounds = [0]
    edge_bounds = [0]
    for c in range(1, NCORES):
        pos = min(int(round(c * E / NCORES)), E - 1)
        node_c = max(int(si_all[pos]), node_bounds[-1] + 1)
        node_bounds.append(node_c)
        edge_bounds.append(int(np.searchsorted(si_all, node_c)))
    node_bounds.append(N)
    edge_bounds.append(E)
    NSLM = max(node_bounds[c + 1] - node_bounds[c] for c in range(NCORES))
    NBLK = math.ceil(NSLM / 128)

    # ---- per-core edge organization ----
    WIN = 96
    core_data = []
    for c in range(NCORES):
        s, e = edge_bounds[c], edge_bounds[c + 1]
        nb = node_bounds[c]
        si = si_all[s:e]
        sj = sj_all[s:e]
        sb = sb_all[s:e]
        jb = sj // JB
        sub = np.lexsort((si, jb))
        si, sj, sb, jb = si[sub], sj[sub], sb[sub], jb[sub]
        jb_starts = [int(np.searchsorted(jb, b)) for b in range(NJB)] + [len(jb)]

        per_jb = []
        for b in range(NJB):
            lo, hi = jb_starts[b], jb_starts[b + 1]
            tiles = []  # (estart, ecount, first_node)
            if hi > lo:
                nodes, counts = np.unique(si[lo:hi], return_counts=True)
                estart = lo + np.concatenate([[0], np.cumsum(counts)[:-1]])
                cur = None
                for k in range(len(nodes)):
                    d = int(counts[k])
                    assert d <= TILE
                    n0 = int(nodes[k])
                    if (cur is None or cur[1] + d > TILE
                            or n0 - cur[2] >= WIN):
                        if cur is not None:
                            tiles.append(tuple(cur))
                        cur = [int(estart[k]), 0, n0]
                    cur[1] += d
                if cur is not None:
                    tiles.append(tuple(cur))
            per_jb.append(tiles)
        core_data.append(dict(nb=nb, si=si, sj=sj, sb=sb, per_jb=per_jb))

    NTJB = max(len(cd["per_jb"][b]) for cd in core_data for b in range(NJB))
    NGJB = math.ceil(math.ceil(NTJB / TPG) / GB) * GB
    NTJB = NGJB * TPG
    assert WIN * (NTJB + 1) <= 32767, (WIN, NTJB)
    NG = NGJB * NJB
    NGB = NG // GB  # gather batches (all groups of a batch share one jb)

    NSI = math.ceil(NBLK * 128 / 1024)
    NASG = math.ceil(N / 1024)
    NAS = NSI + NASG
    NAS = math.ceil(NAS / AB) * AB
    NBAT = math.ceil(NBLK * 128 / GRP)
    NOUT = NBAT * GRP
    # phase C gather split per jb: two instructions
    C1 = min(8192, NOUT)
    C2 = NOUT - C1

    per_core = []
    for c in range(NCORES):
        cd = core_data[c]
        nb, si, sj, sb = cd["nb"], cd["si"], cd["sj"], cd["sb"]

        meta = np.zeros((NGB, 128, 2 * MWB), np.int16)
        loc = np.full((NG, 128, TPG), PAD_LOC, NPF)
        bas_g = np.zeros((NG, 128, 4 * TILE), NPF)

        for b in range(NJB):
            tiles = cd["per_jb"][b]
            for qb in range(NGJB // GB):
                gi_lin = np.zeros((BATCH,), np.int16)
                gj_lin = np.zeros((BATCH,), np.int16)
                for gg in range(GB):
                    g = qb * GB + gg
                    gidx = b * NGJB + g
                    for t in range(TPG):
                        ti = g * TPG + t
                        if ti >= len(tiles):
                            continue
                        es, cnt, fn = tiles[ti]
                        if cnt == 0:
                            continue
                        o = gg * GRP + t * TILE
                        gi_lin[o:o + cnt] = (si[es:es + cnt] - nb
                                             ).astype(np.int16)
                        gj_lin[o:o + cnt] = (sj[es:es + cnt] - JB * b
                                             ).astype(np.int16)
                        loc[gidx, :cnt, t] = (si[es:es + cnt] - fn
                                              ).astype(NPF)
                        kk, h = t // 2, t % 2
                        bas_g[gidx, 64 * h:64 * h + 64,
                              128 * kk:128 * kk + cnt] = sb[es:es + cnt].T
                bidx = b * (NGJB // GB) + qb
                meta[bidx, :, :MWB] = _wrap16(gi_lin)
                meta[bidx, :, MWB:] = _wrap16(gj_lin)

        # phase C: fidx[jb] -> stage row (t*WIN + w) or the zeroed dump row
        fidx = np.zeros((NJB, 128, NOUT // 16), np.int16)
        for b in range(NJB):
            tiles = cd["per_jb"][b]
            node2row = np.full((NOUT,), NTJB * WIN, np.int32)
            for ti, (es, cnt, fn) in enumerate(tiles):
                if cnt == 0:
                    continue
                nn = np.unique(si[es:es + cnt])
                node2row[nn - nb] = ti * WIN + (nn - fn)
            n2r = node2row.astype(np.int16)
            fidx[b, :, :C1 // 16] = _wrap16(n2r[:C1])
            if C2:
                fidx[b, :, C1 // 16:] = _wrap16(n2r[C1:])

        # phase A input packing (stacked pairs)
        p1s = np.zeros((NAS, 128, 512), NPF)
        rows_pad = np.zeros((NAS * 1024, 64), NPF)
        for st in range(NSI):
            g0 = nb + 1024 * st
            g1 = min(g0 + 1024, N)
            if g1 > g0:
                rows_pad[st * 1024: st * 1024 + (g1 - g0)] = p1[g0:g1]
        for st in range(NASG):
            g0 = 1024 * st
            g1 = min(g0 + 1024, N)
            rows_pad[(NSI + st) * 1024: (NSI + st) * 1024 + (g1 - g0)] = \
                p1[g0:g1]
        r4 = rows_pad.reshape(NAS, 4, 2, 128, 64)  # st, u, s, p, f
        for u in range(4):
            for sg in range(2):
                p1s[:, 64 * sg:64 * sg + 64, 128 * u:128 * u + 128] = \
                    r4[:, u, sg].transpose(0, 2, 1)

        per_core.append(dict(
            p1s=p1s.astype(NPB),
            bas_g=bas_g.astype(NPB),
            meta=meta,
            loc=loc.astype(NPB),
            fidx=fidx,
        ))

    w = weights
    W_mid = (w["pi_w2"] @ w["ii_w1"]).astype(NPF)
    b_mid = (w["pi_b2"] @ w["ii_w1"] + w["ii_b1"]).astype(NPF)

    def stack_b(b):
        return np.concatenate([b, b]).reshape(128, 1).astype(NPF)

    consts = dict(
        w1pp_bd=_bd(w["pp_w1"].astype(NPF)).astype(NPB),
        w2pp_bd=_bd(w["pp_w2"].astype(NPF)).astype(NPB),
        w1pi_bd=_bd(w["pi_w1"].astype(NPF)).astype(NPB),
        wmid_bd=_bd(W_mid).astype(NPB),
        w2ii_bd=_bd(w["ii_w2"].astype(NPF)).astype(NPB),
        ident=np.eye(128, dtype=NPB),
        iota=np.tile(np.arange(WIN, dtype=NPF), (128, TPG)).astype(NPB),
        b_pp1=stack_b(w["pp_b1"]),
        b_pi1=stack_b(w["pi_b1"]),
        b_mid=stack_b(b_mid.reshape(-1)),
        ones_row=np.ones((1, 128), NPB),
        bpp2_row=np.tile(w["pp_b2"], 2).reshape(1, 2 * D).astype(NPB),
        bii2_row=np.tile(w["ii_b2"], 2).reshape(1, 2 * D).astype(NPB),
        zrow=np.zeros((1, D), NPB),
    )

    dims = dict(N=N, E=E, NTJB=NTJB, NGJB=NGJB, NG=NG, NGB=NGB, WIN=WIN,
                NSI=NSI, NAS=NAS, NASG=NASG, NBLK=NBLK, NBAT=NBAT,
                NOUT=NOUT, C1=C1, C2=C2, node_bounds=node_bounds)
    return per_core, consts, dims


# ------------------------------------------------------------- device build

def build(nc, dims, consts):
    NTJB, NGJB, NG, NGB = dims["NTJB"], dims["NGJB"], dims["NG"], dims["NGB"]
    WIN, NSI, NAS = dims["WIN"], dims["NSI"], dims["NAS"]
    NOUT, C1, C2 = dims["NOUT"], dims["C1"], dims["C2"]
    has_bpp1 = bool(np.any(consts["b_pp1"] != 0))
    has_bpp2 = bool(np.any(consts["bpp2_row"].astype(NPF) != 0))
    has_bpi1 = bool(np.any(consts["b_pi1"] != 0))
    has_bmid = bool(np.any(consts["b_mid"] != 0))
    has_bii2 = bool(np.any(consts["bii2_row"].astype(NPF) != 0))

    t_p1s = nc.dram_tensor("p1s", (NAS, 128, 512), BF, kind="ExternalInput")
    t_bas = nc.dram_tensor("bas_g", (NG, 128, 512), BF, kind="ExternalInput")
    t_meta = nc.dram_tensor("meta", (NGB, 128, 2 * MWB), I16,
                            kind="ExternalInput")
    t_loc = nc.dram_tensor("loc", (NG, 128, TPG), BF, kind="ExternalInput")
    t_fidx = nc.dram_tensor("fidx", (NJB, 128, NOUT // 16), I16,
                            kind="ExternalInput")
    cts = {}
    cdt = dict(b_pp1=FP, b_pi1=FP, b_mid=FP)
    for nm in ["w1pp_bd", "w2pp_bd", "w1pi_bd", "wmid_bd", "w2ii_bd",
               "ident", "iota", "b_pp1", "b_pi1", "b_mid", "ones_row",
               "bpp2_row", "bii2_row", "zrow"]:
        cts[nm] = nc.dram_tensor(nm, consts[nm].shape, cdt.get(nm, BF),
                                 kind="ExternalInput")
    t_out = nc.dram_tensor("out", (NOUT, D), FP, kind="ExternalOutput")

    jtab = [nc.dram_tensor(f"jtab{b}", (JB, 128), BF, kind="Internal")
            for b in range(NJB)]
    itab = nc.dram_tensor("itab", (NSI * 1024, 128), BF, kind="Internal")
    stage = [nc.dram_tensor(f"stage{b}", (NTJB + 1, WIN, 128), BF,
                            kind="Internal")
             for b in range(NJB)]

    def load_consts(pool):
        sb = {}
        for nm, t in cts.items():
            tile = pool.tile(list(consts[nm].shape), cdt.get(nm, BF), tag=nm)
            nc.sync.dma_start(tile[:], t[:])
            sb[nm] = tile
        return sb

    Tanh = mybir.ActivationFunctionType.Tanh
    Copy = mybir.ActivationFunctionType.Copy

    def mm(out, lhsT, rhs, **kw):
        nc.tensor.matmul(out, lhsT=lhsT, rhs=rhs, **kw)

    _PH = os.environ.get("GC_PHASES", "ABC")

    # ---------------- phase A: pp1 tables ----------------
    if "A" in _PH:
      with TileContext(nc) as tc:
        with tc.tile_pool(name="cstA", bufs=1) as cpool, \
             tc.tile_pool(name="ldA", bufs=2) as lpool, \
             tc.tile_pool(name="sbA", bufs=3) as pool, \
             tc.tile_pool(name="psA", bufs=2, space="PSUM") as psA, \
             tc.tile_pool(name="psA2", bufs=2, space="PSUM") as psA2:
            sbk = load_consts(cpool)
            for sb0 in range(0, NAS, AB):
                p1c = lpool.tile([128, AB * 512], BF, tag="p1c")
                nc.sync.dma_start(
                    p1c[:].rearrange("p (a c) -> p a c", a=AB),
                    t_p1s[sb0:sb0 + AB].rearrange("a p c -> p a c"))
                for a in range(AB):
                    st = sb0 + a
                    ps1 = psA.tile([128, 512], FP, tag="ps1")
                    mm(ps1[:], lhsT=sbk["w1pp_bd"][:],
                       rhs=p1c[:, 512 * a:512 * a + 512],
                       start=True, stop=True)
                    h1 = pool.tile([128, 512], BF, tag="h1a")
                    if has_bpp1:
                        nc.scalar.activation(h1[:], ps1[:], Tanh,
                                             bias=sbk["b_pp1"][:])
                    else:
                        nc.scalar.activation(h1[:], ps1[:], Tanh)
                    ps2 = psA2.tile([128, 512], FP, tag="ps2")
                    for u in range(4):
                        mm(ps2[:, 128 * u:128 * u + 128],
                           lhsT=h1[:, 128 * u:128 * u + 128],
                           rhs=sbk["w2pp_bd"][:], start=True,
                           stop=not has_bpp2)
                        if has_bpp2:
                            mm(ps2[:, 128 * u:128 * u + 128],
                               lhsT=sbk["ones_row"][:, :],
                               rhs=sbk["bpp2_row"][:, :],
                               start=False, stop=True)
                    tsb = pool.tile([128, 512], BF, tag="tsb")
                    nc.vector.tensor_copy(tsb[:], ps2[:])
                    if st < NSI:
                        dst = itab[1024 * st:1024 * (st + 1), 0:64]
                    else:
                        g0 = (st - NSI) * 1024
                        b = min(g0 // JB, NJB - 1)
                        r0 = g0 - b * JB
                        dst = jtab[b][r0:r0 + 1024, 0:64]
                    nc.sync.dma_start(
                        dst.rearrange("(b p) f -> p b f", p=128),
                        tsb[:].rearrange("p (b f) -> p b f", b=8))

    # ---------------- phase B: edges ----------------
    if "B" in _PH:
      with TileContext(nc) as tc:
        with tc.tile_pool(name="cstB", bufs=1) as cpool, \
             tc.tile_pool(name="meta", bufs=2) as mpool, \
             tc.tile_pool(name="gat", bufs=2) as gpool, \
             tc.tile_pool(name="sbB", bufs=3) as pool, \
             tc.tile_pool(name="psT", bufs=2, space="PSUM") as psT, \
             tc.tile_pool(name="psH", bufs=1, space="PSUM") as psH, \
             tc.tile_pool(name="psE", bufs=2, space="PSUM") as psE, \
             tc.tile_pool(name="psS", bufs=2, space="PSUM") as psS:
            sbk = load_consts(cpool)
            # zero the dedicated dump row of every stage tensor
            for b in range(NJB):
                srows = stage[b][:].rearrange("t w f -> (t w) f")
                nc.sync.dma_start(
                    srows[NTJB * WIN:NTJB * WIN + 1, 0:64], sbk["zrow"][:])
            for bidx in range(NGB):
                q0 = bidx * GB
                b = q0 // NGJB
                mt = mpool.tile([128, 2 * MWB], I16, tag="mt")
                nc.sync.dma_start(mt[:], t_meta[bidx])
                lc = mpool.tile([128, GB * TPG], BF, tag="lc")
                nc.sync.dma_start(
                    lc[:].rearrange("p (q c) -> p q c", q=GB),
                    t_loc[q0:q0 + GB].rearrange("q p c -> p q c"))
                bas4 = mpool.tile([128, GB * 512], BF, tag="bas4")
                nc.sync.dma_start(
                    bas4[:].rearrange("p (q c) -> p q c", q=GB),
                    t_bas[q0:q0 + GB].rearrange("q p c -> p q c"))

                gi = gpool.tile([128, GB * TPG, 64], BF, tag="gi")
                dma_gather_raw(
                    nc, gi[:], itab[:, 0:64], mt[0:16, 0:MWB],
                    num_idxs=BATCH, elem_size=64, elem_step=128,
                    queue_num=0)
                gj = gpool.tile([128, GB * TPG, 64], BF, tag="gj")
                dma_gather_raw(
                    nc, gj[:], jtab[b][:, 0:64], mt[0:16, MWB:2 * MWB],
                    num_idxs=BATCH, elem_size=64, elem_step=128,
                    queue_num=1)

                for qq in range(GB):
                    gidx = q0 + qq
                    g = gidx - b * NGJB
                    bas = bas4[:, qq * 512:qq * 512 + 512]

                    gsum = pool.tile([128, 512], BF, tag="gsum")
                    nc.vector.tensor_tensor(
                        out=gsum[:].rearrange("p (b f) -> p b f", b=TPG),
                        in0=gi[:, qq * TPG:(qq + 1) * TPG, :],
                        in1=gj[:, qq * TPG:(qq + 1) * TPG, :],
                        op=mybir.AluOpType.add)

                    pst = psT.tile([128, 512], BF, tag="pst")
                    for kk in range(4):
                        mm(pst[:, 128 * kk:128 * kk + 128],
                           lhsT=gsum[:, 128 * kk:128 * kk + 128],
                           rhs=sbk["ident"][:], is_transpose=True,
                           start=True, stop=True)
                    interf = pool.tile([128, 512], BF, tag="interf")
                    nc.vector.tensor_tensor(out=interf[:], in0=pst[:],
                                            in1=bas[:],
                                            op=mybir.AluOpType.add)

                    ph1 = psH.tile([128, 512], FP, tag="ph1")
                    mm(ph1[:], lhsT=sbk["w1pi_bd"][:], rhs=interf[:],
                       start=True, stop=True)
                    h1 = pool.tile([128, 512], BF, tag="h1")
                    if has_bpi1:
                        nc.scalar.activation(h1[:], ph1[:], Tanh,
                                             bias=sbk["b_pi1"][:])
                    else:
                        nc.scalar.activation(h1[:], ph1[:], Tanh)

                    ph2 = psH.tile([128, 512], FP, tag="ph2")
                    mm(ph2[:], lhsT=sbk["wmid_bd"][:], rhs=h1[:],
                       start=True, stop=True)
                    h2 = pool.tile([128, 512], BF, tag="h2")
                    if has_bmid:
                        nc.scalar.activation(h2[:], ph2[:], Tanh,
                                             bias=sbk["b_mid"][:])
                    else:
                        nc.scalar.activation(h2[:], ph2[:], Tanh)

                    pse = psE.tile([128, 512], FP, tag="pse")
                    for kk in range(4):
                        mm(pse[:, 128 * kk:128 * kk + 128],
                           lhsT=h2[:, 128 * kk:128 * kk + 128],
                           rhs=sbk["w2ii_bd"][:], start=True,
                           stop=not has_bii2)
                        if has_bii2:
                            mm(pse[:, 128 * kk:128 * kk + 128],
                               lhsT=sbk["ones_row"][:, :],
                               rhs=sbk["bii2_row"][:, :],
                               start=False, stop=True)
                    iiem = pool.tile([128, 512], BF, tag="iiem")
                    nc.vector.tensor_copy(iiem[:], pse[:])

                    oh = pool.tile([128, TPG * WIN], BF, tag="oh")
                    nc.vector.tensor_tensor(
                        out=oh[:].rearrange("p (b w) -> p b w", b=TPG),
                        in0=lc[:, qq * TPG:(qq + 1) * TPG]
                              .to_broadcast([128, TPG, WIN]),
                        in1=sbk["iota"][:].rearrange("p (b w) -> p b w",
                                                     b=TPG),
                        op=mybir.AluOpType.is_equal)

                    pss = psS.tile([WIN, 512], FP, tag="pss")
                    for t in range(TPG):
                        mm(pss[:, 64 * t:64 * t + 64],
                           lhsT=oh[:, WIN * t:WIN * t + WIN],
                           rhs=iiem[:, 64 * t:64 * t + 64],
                           start=True, stop=True)
                    s_sb = pool.tile([WIN, 512], BF, tag="s_sb")
                    nc.scalar.activation(s_sb[:], pss[:], Copy)
                    nc.sync.dma_start(
                        stage[b][TPG * g:TPG * (g + 1), :, 0:64]
                            .rearrange("t w f -> w t f"),
                        s_sb[:].rearrange("w (t f) -> w t f", t=TPG))

    # ---------------- phase C: merge stages -> out ----------------
    NB1 = C1 // 128
    NB2 = C2 // 128
    if "C" in _PH:
      with TileContext(nc) as tc:
        with tc.tile_pool(name="metaC", bufs=1) as mpool, \
             tc.tile_pool(name="gatC", bufs=1) as gpool, \
             tc.tile_pool(name="sbC", bufs=1) as pool:
            slabs = []
            for b in range(NJB):
                fx = mpool.tile([128, NOUT // 16], I16, tag=f"fx{b}")
                nc.sync.dma_start(fx[:], t_fidx[b])
                sl = gpool.tile([128, NOUT // 128, D], BF, tag=f"sl{b}")
                srows = stage[b][:].rearrange("t w f -> (t w) f")[:, 0:64]
                dma_gather_raw(
                    nc, sl[:, 0:NB1, :], srows, fx[0:16, 0:C1 // 16],
                    num_idxs=C1, elem_size=D, elem_step=128,
                    queue_num=b % 2)
                if C2:
                    dma_gather_raw(
                        nc, sl[:, NB1:NB1 + NB2, :], srows,
                        fx[0:16, C1 // 16:],
                        num_idxs=C2, elem_size=D, elem_step=128,
                        queue_num=b % 2)
                slabs.append(sl)
            acc01 = pool.tile([128, (NOUT // 128) * D], BF, tag="acc01")
            nc.vector.tensor_tensor(
                out=acc01[:].rearrange("p (b f) -> p b f", b=NOUT // 128),
                in0=slabs[0][:], in1=slabs[1][:], op=mybir.AluOpType.add)
            acc23 = pool.tile([128, (NOUT // 128) * D], BF, tag="acc23")
            nc.vector.tensor_tensor(
                out=acc23[:].rearrange("p (b f) -> p b f", b=NOUT // 128),
                in0=slabs[2][:], in1=slabs[3][:], op=mybir.AluOpType.add)
            accf = pool.tile([128, (NOUT // 128) * D], FP, tag="accf")
            nc.vector.tensor_tensor(out=accf[:], in0=acc01[:],
                                    in1=acc23[:], op=mybir.AluOpType.add)
            nc.sync.dma_start(
                t_out[:].rearrange("(b p) f -> p b f", p=128),
                accf[:].rearrange("p (b f) -> p b f", b=NOUT // 128))
    nc.compile()


# ----------------------------------------------------------------- kernel()

SHARED_NAMES = ["w1pp_bd", "w2pp_bd", "w1pi_bd", "wmid_bd", "w2ii_bd",
                "ident", "iota", "b_pp1", "b_pi1", "b_mid", "ones_row",
                "bpp2_row", "bii2_row", "zrow"]
PER_CORE_NAMES = ["p1s", "bas_g", "meta", "loc", "fidx"]


def make_in_maps(per_core, consts):
    shared = {nm: consts[nm] for nm in SHARED_NAMES}
    in_maps = []
    for c in range(NCORES):
        m = dict(shared)
        for nm in PER_CORE_NAMES:
            m[nm] = per_core[c][nm]
        in_maps.append(m)
    return in_maps


def kernel(**inputs):
    idx_i = np.asarray(inputs["idx_i"]).astype(np.int64)
    idx_j = np.asarray(inputs["idx_j"]).astype(np.int64)
    p1 = np.asarray(inputs["p1"], dtype=NPF)
    basis = np.asarray(inputs["basis"], dtype=NPF)
    weights = {k: np.asarray(inputs[k], dtype=NPF) for k in
               ["pp_w1", "pp_b1", "pp_w2", "pp_b2",
                "pi_w1", "pi_b1", "pi_w2", "pi_b2",
                "ii_w1", "ii_b1", "ii_w2", "ii_b2"]}

    per_core, consts, dims = prep(idx_i, idx_j, p1, basis, weights)

    nc = make_nc()
    build(nc, dims, consts)

    res = run_bass_kernel_spmd(nc, make_in_maps(per_core, consts),
                               core_ids=list(range(NCORES)))
    global LAST_EXEC_NS
    LAST_EXEC_NS = res.exec_time_ns

    N = dims["N"]
    nbs = dims["node_bounds"]
    out = np.zeros((N, D), dtype=NPF)
    for c in range(NCORES):
        out[nbs[c]:nbs[c + 1]] = res.results[c]["out"][:nbs[c + 1] - nbs[c]]
    deg = np.bincount(idx_i, minlength=N)
    out[deg == 0] = 0
    return out


# revision 20
# speedup vs baseline: 6.9365x; 1.0146x over previous
"""GCBlock GNN message-passing kernel for 8 Trainium2 NeuronCores.

Strategy (v5 — bulk int16 dma_gather, 4096-row batches):
  * Host: shard edges by destination node range (each core owns a disjoint
    output range -> no collectives). Within a core, sort edges by
    (j-block, i) where j-blocks are 25600-node ranges, so that j-gather
    indices are block-local int16 and i-gather indices are core-local int16.
    Pack edges into 128-edge tiles of whole nodes with node span < 96.
  * Device phase A: compute the pp1 = MLP(p1) node table into DRAM twice:
    an i-table holding this core's node range and four j-block tables,
    bf16 rows padded to 256B (the gather stride field is in 256B units).
    Inputs arrive host-packed in stacked-pair FM layout ([128,512] = two
    64-feature panels on the partition axis) so matmuls/tanh run with
    block-diagonal weights at full 128-partition width.
  * Device phase B (per 4-group batch = 4096 edges): ONE 4096-row
    dma_gather for i-rows + ONE for j-rows (994ns fixed SWDGE cost
    amortized over 4096 descriptors; descriptors read only the 128B
    payload of each 256B row). Per 1024-edge group: DVE add, PE transposes
    to stacked FM, DVE basis add, 3 matmul layers with block-diagonal
    weights, tanh on ScalarE, one-hot scatter matmuls into per-tile
    96-row windows, one psum->sbuf copy and ONE static write of all 8
    windows into a tile-major bf16 stage tensor.
  * Device phase C: per j-block, TWO dma_gathers fetch every output row's
    stage partial; 3 DVE adds + one static write produce the final
    segment sums.
  * All data-dependent structure lives in index tensors; the instruction
    schedule is identical across cores (SPMD single program).
"""

import math
import os

import numpy as np
import ml_dtypes

import concourse.bacc as bacc
import concourse.bass as bass
import concourse.mybir as mybir
from concourse.bass_utils import run_bass_kernel_spmd
from concourse.tile import TileContext

D = 64
TILE = 128            # edges per tile
TPG = 8               # tiles per group
GRP = TILE * TPG      # 1024 edges per group
GB = 4                # groups per gather batch (4096 edges)
BATCH = GRP * GB
MWB = BATCH // 16     # idx columns after 16-partition wrap (256)
NCORES = 8
JB = 25600            # j-block size (int16-safe, multiple of 1024)
NJB = 4
PAD_LOC = 300.0       # one-hot local index for pad edges (matches nothing)
AB = 2                # phase-A steps per load batch

FP = mybir.dt.float32
BF = mybir.dt.bfloat16
I16 = mybir.dt.int16
NPF = np.float32
NPB = ml_dtypes.bfloat16


def make_nc():
    return bacc.Bacc(trn_type="TRN2", num_swdge_queues=2)


def dma_gather_raw(nc, out_ap, in_ap, idxs_ap, num_idxs, elem_size,
                   elem_step, queue_num=0):
    """dma_gather without the helper's 256B elem minimum / 1024-idx packet.

    The ISA stride field is in 256B units (stride must be %256), but the
    per-descriptor read size is free — reading the 128B payload of padded
    256B rows halves DMA-engine time vs. gathering the full padded row.
    single_packet=False lets num_idxs exceed the 1024-descriptor ring.
    """
    from concourse import ap_utils
    g = nc.gpsimd
    assert idxs_ap.dtype == I16
    assert in_ap.dtype == out_ap.dtype
    stride_bytes = elem_step * mybir.dt.size(in_ap.dtype)
    stride_bytes_256 = stride_bytes // 256
    assert stride_bytes_256 * 256 == stride_bytes and stride_bytes_256 < 256
    assert ap_utils.ap_is_contiguous(out_ap.ap[1:])
    assert ap_utils.ap_is_contiguous(idxs_ap.ap[1:])
    assert in_ap.ap[0][0] == elem_step
    assert in_ap.ap[-1][1] == elem_size
    assert out_ap.ap[-1][1] == elem_size
    _in_ap = g.lower_ap_dma(in_ap, for_custom_bir_dma=True)
    _idxs_ap = g.lower_ap(idxs_ap)
    _out_ap = g.lower_ap(out_ap)
    return g.add_instruction(
        mybir.InstDMAGatherAnt(
            name=g.bass.get_next_instruction_name(),
            ins=[*_in_ap, _idxs_ap, g.lower_val_access(g.to_reg(num_idxs))],
            outs=[_out_ap],
            transpose=False,
            num_idxs=num_idxs,
            elem_size=elem_size,
            stride_bytes_256=stride_bytes_256,
            gen_mode=0,
            single_packet=False,
            queue_num=queue_num,
            sbuf_tokens_per_rank=0,
            sbuf_free_dim_per_rank=0,
            sbuf_free_dim_pad_per_rank=0,
            sbuf_byte_offset=0,
        ))


def _wrap16(lin):
    """[n] int16 linear index list -> [128, n//16] SWDGE-wrapped+replicated."""
    n = lin.shape[0]
    w = lin.reshape(n // 16, 16).T
    return np.tile(w, (8, 1)).copy()


def _bd(w):
    """64x64 -> 128x128 block-diagonal (stacked-pair weights)."""
    out = np.zeros((128, 128), dtype=w.dtype)
    out[:64, :64] = w
    out[64:, 64:] = w
    return out


# ---------------------------------------------------------------- host prep

def prep(idx_i, idx_j, p1, basis, weights):
    N, E = p1.shape[0], idx_i.shape[0]
    assert N <= NJB * JB

    order = np.argsort(idx_i, kind="stable")
    si_all = idx_i[order]
    sj_all = idx_j[order]
    sb_all = basis[order]

    # core boundaries snapped to node edges, balancing edge counts
    node_bounds = [0]
    edge_bounds = [0]
    for c in range(1, NCORES):
        pos = min(int(round(c * E / NCORES)), E - 1)
        node_c = max(int(si_all[pos]), node_bounds[-1] + 1)
        node_bounds.append(node_c)
        edge_bounds.append(int(np.searchsorted(si_all, node_c)))
    node_bounds.append(N)
    edge_bounds.append(E)
    NSLM = max(node_bounds[c + 1] - node_bounds[c] for c in range(NCORES))
    NBLK = math.ceil(NSLM / 128)

    # ---- per-core edge organization ----
    WIN = 96
    core_data = []
    for c in range(NCORES):
        s, e = edge_bounds[c], edge_bounds[c + 1]
        nb = node_bounds[c]
        si = si_all[s:e]
        sj = sj_all[s:e]
        sb = sb_all[s:e]
        jb = sj // JB
        sub = np.lexsort((si, jb))
        si, sj, sb, jb = si[sub], sj[sub], sb[sub], jb[sub]
        jb_starts = [int(np.searchsorted(jb, b)) for b in range(NJB)] + [len(jb)]

        per_jb = []
        for b in range(NJB):
            lo, hi = jb_starts[b], jb_starts[b + 1]
            tiles = []  # (estart, ecount, first_node)
            if hi > lo:
                nodes, counts = np.unique(si[lo:hi], return_counts=True)
                estart = lo + np.concatenate([[0], np.cumsum(counts)[:-1]])
                cur = None
                for k in range(len(nodes)):
                    d = int(counts[k])
                    assert d <= TILE
                    n0 = int(nodes[k])
                    if (cur is None or cur[1] + d > TILE
                            or n0 - cur[2] >= WIN):
                        if cur is not None:
                            tiles.append(tuple(cur))
                        cur = [int(estart[k]), 0, n0]
                    cur[1] += d
                if cur is not None:
                    tiles.append(tuple(cur))
            per_jb.append(tiles)
        core_data.append(dict(nb=nb, si=si, sj=sj, sb=sb, per_jb=per_jb))

    NTJB = max(len(cd["per_jb"][b]) for cd in core_data for b in range(NJB))
    NGJB = math.ceil(math.ceil(NTJB / TPG) / GB) * GB
    NTJB = NGJB * TPG
    assert WIN * (NTJB + 1) <= 32767, (WIN, NTJB)
    NG = NGJB * NJB
    NGB = NG // GB  # gather batches (all groups of a batch share one jb)

    NSI = math.ceil(NBLK * 128 / 1024)
    NASG = math.ceil(N / 1024)
    NAS = NSI + NASG
    NAS = math.ceil(NAS / AB) * AB
    NBAT = math.ceil(NBLK * 128 / GRP)
    NOUT = NBAT * GRP
    # phase C gather split per jb: two instructions
    C1 = NOUT
    C2 = 0

    per_core = []
    for c in range(NCORES):
        cd = core_data[c]
        nb, si, sj, sb = cd["nb"], cd["si"], cd["sj"], cd["sb"]

        meta = np.zeros((NGB, 128, 2 * MWB), np.int16)
        loc = np.full((NG, 128, TPG), PAD_LOC, NPF)
        bas_g = np.zeros((NG, 128, 4 * TILE), NPF)

        for b in range(NJB):
            tiles = cd["per_jb"][b]
            for qb in range(NGJB // GB):
                gi_lin = np.zeros((BATCH,), np.int16)
                gj_lin = np.zeros((BATCH,), np.int16)
                for gg in range(GB):
                    g = qb * GB + gg
                    gidx = b * NGJB + g
                    for t in range(TPG):
                        ti = g * TPG + t
                        if ti >= len(tiles):
                            continue
                        es, cnt, fn = tiles[ti]
                        if cnt == 0:
                            continue
                        o = gg * GRP + t * TILE
                        gi_lin[o:o + cnt] = (si[es:es + cnt] - nb
                                             ).astype(np.int16)
                        gj_lin[o:o + cnt] = (sj[es:es + cnt] - JB * b
                                             ).astype(np.int16)
                        loc[gidx, :cnt, t] = (si[es:es + cnt] - fn
                                              ).astype(NPF)
                        kk, h = t // 2, t % 2
                        bas_g[gidx, 64 * h:64 * h + 64,
                              128 * kk:128 * kk + cnt] = sb[es:es + cnt].T
                bidx = b * (NGJB // GB) + qb
                meta[bidx, :, :MWB] = _wrap16(gi_lin)
                meta[bidx, :, MWB:] = _wrap16(gj_lin)

        # phase C: fidx[jb] -> stage row (t*WIN + w) or the zeroed dump row
        fidx = np.zeros((NJB, 128, NOUT // 16), np.int16)
        for b in range(NJB):
            tiles = cd["per_jb"][b]
            node2row = np.full((NOUT,), NTJB * WIN, np.int32)
            for ti, (es, cnt, fn) in enumerate(tiles):
                if cnt == 0:
                    continue
                nn = np.unique(si[es:es + cnt])
                node2row[nn - nb] = ti * WIN + (nn - fn)
            nblocks = NOUT // 128
            r = np.arange(NOUT)
            perm = (r % 128) * nblocks + r // 128
            n2r = node2row[perm].astype(np.int16)
            fidx[b, :, :C1 // 16] = _wrap16(n2r[:C1])
            if C2:
                fidx[b, :, C1 // 16:] = _wrap16(n2r[C1:])

        # phase A input packing (stacked pairs)
        p1s = np.zeros((NAS, 128, 512), NPF)
        rows_pad = np.zeros((NAS * 1024, 64), NPF)
        for st in range(NSI):
            g0 = nb + 1024 * st
            g1 = min(g0 + 1024, N)
            if g1 > g0:
                rows_pad[st * 1024: st * 1024 + (g1 - g0)] = p1[g0:g1]
        for st in range(NASG):
            g0 = 1024 * st
            g1 = min(g0 + 1024, N)
            rows_pad[(NSI + st) * 1024: (NSI + st) * 1024 + (g1 - g0)] = \
                p1[g0:g1]
        r4 = rows_pad.reshape(NAS, 4, 2, 128, 64)  # st, u, s, p, f
        for u in range(4):
            for sg in range(2):
                p1s[:, 64 * sg:64 * sg + 64, 128 * u:128 * u + 128] = \
                    r4[:, u, sg].transpose(0, 2, 1)

        per_core.append(dict(
            p1s=p1s.astype(NPB),
            bas_g=bas_g.astype(NPB),
            meta=meta,
            loc=loc.astype(NPB),
            fidx=fidx,
        ))

    w = weights
    W_mid = (w["pi_w2"] @ w["ii_w1"]).astype(NPF)
    b_mid = (w["pi_b2"] @ w["ii_w1"] + w["ii_b1"]).astype(NPF)

    def stack_b(b):
        return np.concatenate([b, b]).reshape(128, 1).astype(NPF)

    consts = dict(
        w1pp_bd=_bd(w["pp_w1"].astype(NPF)).astype(NPB),
        w2pp_bd=_bd(w["pp_w2"].astype(NPF)).astype(NPB),
        w1pi_bd=_bd(w["pi_w1"].astype(NPF)).astype(NPB),
        wmid_bd=_bd(W_mid).astype(NPB),
        w2ii_bd=_bd(w["ii_w2"].astype(NPF)).astype(NPB),
        ident=np.eye(128, dtype=NPB),
        iota=np.tile(np.arange(WIN, dtype=NPF), (128, TPG)).astype(NPB),
        b_pp1=stack_b(w["pp_b1"]),
        b_pi1=stack_b(w["pi_b1"]),
        b_mid=stack_b(b_mid.reshape(-1)),
        ones_row=np.ones((1, 128), NPB),
        bpp2_row=np.tile(w["pp_b2"], 2).reshape(1, 2 * D).astype(NPB),
        bii2_row=np.tile(w["ii_b2"], 2).reshape(1, 2 * D).astype(NPB),
        zrow=np.zeros((1, D), NPB),
    )

    dims = dict(N=N, E=E, NTJB=NTJB, NGJB=NGJB, NG=NG, NGB=NGB, WIN=WIN,
                NSI=NSI, NAS=NAS, NASG=NASG, NBLK=NBLK, NBAT=NBAT,
                NOUT=NOUT, C1=C1, C2=C2, node_bounds=node_bounds)
    return per_core, consts, dims


# ------------------------------------------------------------- device build

def build(nc, dims, consts):
    NTJB, NGJB, NG, NGB = dims["NTJB"], dims["NGJB"], dims["NG"], dims["NGB"]
    WIN, NSI, NAS = dims["WIN"], dims["NSI"], dims["NAS"]
    NOUT, C1, C2 = dims["NOUT"], dims["C1"], dims["C2"]
    has_bpp1 = bool(np.any(consts["b_pp1"] != 0))
    has_bpp2 = bool(np.any(consts["bpp2_row"].astype(NPF) != 0))
    has_bpi1 = bool(np.any(consts["b_pi1"] != 0))
    has_bmid = bool(np.any(consts["b_mid"] != 0))
    has_bii2 = bool(np.any(consts["bii2_row"].astype(NPF) != 0))

    t_p1s = nc.dram_tensor("p1s", (NAS, 128, 512), BF, kind="ExternalInput")
    t_bas = nc.dram_tensor("bas_g", (NG, 128, 512), BF, kind="ExternalInput")
    t_meta = nc.dram_tensor("meta", (NGB, 128, 2 * MWB), I16,
                            kind="ExternalInput")
    t_loc = nc.dram_tensor("loc", (NG, 128, TPG), BF, kind="ExternalInput")
    t_fidx = nc.dram_tensor("fidx", (NJB, 128, NOUT // 16), I16,
                            kind="ExternalInput")
    cts = {}
    cdt = dict(b_pp1=FP, b_pi1=FP, b_mid=FP)
    for nm in ["w1pp_bd", "w2pp_bd", "w1pi_bd", "wmid_bd", "w2ii_bd",
               "ident", "iota", "b_pp1", "b_pi1", "b_mid", "ones_row",
               "bpp2_row", "bii2_row", "zrow"]:
        cts[nm] = nc.dram_tensor(nm, consts[nm].shape, cdt.get(nm, BF),
                                 kind="ExternalInput")
    t_out = nc.dram_tensor("out", (NOUT, D), FP, kind="ExternalOutput")

    jtab = [nc.dram_tensor(f"jtab{b}", (JB, 128), BF, kind="Internal")
            for b in range(NJB)]
    itab = nc.dram_tensor("itab", (NSI * 1024, 128), BF, kind="Internal")
    stage = [nc.dram_tensor(f"stage{b}", (NTJB + 1, WIN, 128), BF,
                            kind="Internal")
             for b in range(NJB)]

    def load_consts(pool):
        sb = {}
        for nm, t in cts.items():
            tile = pool.tile(list(consts[nm].shape), cdt.get(nm, BF), tag=nm)
            nc.sync.dma_start(tile[:], t[:])
            sb[nm] = tile
        return sb

    Tanh = mybir.ActivationFunctionType.Tanh
    Copy = mybir.ActivationFunctionType.Copy

    def mm(out, lhsT, rhs, **kw):
        nc.tensor.matmul(out, lhsT=lhsT, rhs=rhs, **kw)

    _PH = os.environ.get("GC_PHASES", "ABC")

    # ---------------- phase A: pp1 tables ----------------
    if "A" in _PH:
      with TileContext(nc) as tc:
        with tc.tile_pool(name="cstA", bufs=1) as cpool, \
             tc.tile_pool(name="ldA", bufs=2) as lpool, \
             tc.tile_pool(name="sbA", bufs=3) as pool, \
             tc.tile_pool(name="psA", bufs=2, space="PSUM") as psA, \
             tc.tile_pool(name="psA2", bufs=2, space="PSUM") as psA2:
            sbk = load_consts(cpool)
            for sb0 in range(0, NAS, AB):
                p1c = lpool.tile([128, AB * 512], BF, tag="p1c")
                nc.sync.dma_start(
                    p1c[:].rearrange("p (a c) -> p a c", a=AB),
                    t_p1s[sb0:sb0 + AB].rearrange("a p c -> p a c"))
                for a in range(AB):
                    st = sb0 + a
                    ps1 = psA.tile([128, 512], FP, tag="ps1")
                    mm(ps1[:], lhsT=sbk["w1pp_bd"][:],
                       rhs=p1c[:, 512 * a:512 * a + 512],
                       start=True, stop=True)
                    h1 = pool.tile([128, 512], BF, tag="h1a")
                    if has_bpp1:
                        nc.scalar.activation(h1[:], ps1[:], Tanh,
                                             bias=sbk["b_pp1"][:])
                    else:
                        nc.scalar.activation(h1[:], ps1[:], Tanh)
                    ps2 = psA2.tile([128, 512], FP, tag="ps2")
                    for u in range(4):
                        mm(ps2[:, 128 * u:128 * u + 128],
                           lhsT=h1[:, 128 * u:128 * u + 128],
                           rhs=sbk["w2pp_bd"][:], start=True,
                           stop=not has_bpp2)
                        if has_bpp2:
                            mm(ps2[:, 128 * u:128 * u + 128],
                               lhsT=sbk["ones_row"][:, :],
                               rhs=sbk["bpp2_row"][:, :],
                               start=False, stop=True)
                    tsb = pool.tile([128, 512], BF, tag="tsb")
                    nc.vector.tensor_copy(tsb[:], ps2[:])
                    if st < NSI:
                        dst = itab[1024 * st:1024 * (st + 1), 0:64]
                    else:
                        g0 = (st - NSI) * 1024
                        b = min(g0 // JB, NJB - 1)
                        r0 = g0 - b * JB
                        dst = jtab[b][r0:r0 + 1024, 0:64]
                    nc.sync.dma_start(
                        dst.rearrange("(b p) f -> p b f", p=128),
                        tsb[:].rearrange("p (b f) -> p b f", b=8))

    # ---------------- phase B: edges ----------------
    if "B" in _PH:
      with TileContext(nc) as tc:
        with tc.tile_pool(name="cstB", bufs=1) as cpool, \
             tc.tile_pool(name="meta", bufs=2) as mpool, \
             tc.tile_pool(name="gat", bufs=2) as gpool, \
             tc.tile_pool(name="sbB", bufs=3) as pool, \
             tc.tile_pool(name="psT", bufs=2, space="PSUM") as psT, \
             tc.tile_pool(name="psH", bufs=1, space="PSUM") as psH, \
             tc.tile_pool(name="psE", bufs=2, space="PSUM") as psE, \
             tc.tile_pool(name="psS", bufs=2, space="PSUM") as psS:
            sbk = load_consts(cpool)
            # zero the dedicated dump row of every stage tensor
            for b in range(NJB):
                srows = stage[b][:].rearrange("t w f -> (t w) f")
                nc.sync.dma_start(
                    srows[NTJB * WIN:NTJB * WIN + 1, 0:64], sbk["zrow"][:])
            for bidx in range(NGB):
                q0 = bidx * GB
                b = q0 // NGJB
                mt = mpool.tile([128, 2 * MWB], I16, tag="mt")
                nc.sync.dma_start(mt[:], t_meta[bidx])
                lc = mpool.tile([128, GB * TPG], BF, tag="lc")
                nc.sync.dma_start(
                    lc[:].rearrange("p (q c) -> p q c", q=GB),
                    t_loc[q0:q0 + GB].rearrange("q p c -> p q c"))
                bas4 = mpool.tile([128, GB * 512], BF, tag="bas4")
                nc.sync.dma_start(
                    bas4[:].rearrange("p (q c) -> p q c", q=GB),
                    t_bas[q0:q0 + GB].rearrange("q p c -> p q c"))

                gi = gpool.tile([128, GB * TPG, 64], BF, tag="gi")
                dma_gather_raw(
                    nc, gi[:], itab[:, 0:64], mt[0:16, 0:MWB],
                    num_idxs=BATCH, elem_size=64, elem_step=128,
                    queue_num=0)
                gj = gpool.tile([128, GB * TPG, 64], BF, tag="gj")
                dma_gather_raw(
                    nc, gj[:], jtab[b][:, 0:64], mt[0:16, MWB:2 * MWB],
                    num_idxs=BATCH, elem_size=64, elem_step=128,
                    queue_num=1)

                for qq in range(GB):
                    gidx = q0 + qq
                    g = gidx - b * NGJB
                    bas = bas4[:, qq * 512:qq * 512 + 512]

                    gsum = pool.tile([128, 512], BF, tag="gsum")
                    nc.vector.tensor_tensor(
                        out=gsum[:].rearrange("p (b f) -> p b f", b=TPG),
                        in0=gi[:, qq * TPG:(qq + 1) * TPG, :],
                        in1=gj[:, qq * TPG:(qq + 1) * TPG, :],
                        op=mybir.AluOpType.add)

                    pst = psT.tile([128, 512], BF, tag="pst")
                    for kk in range(4):
                        mm(pst[:, 128 * kk:128 * kk + 128],
                           lhsT=gsum[:, 128 * kk:128 * kk + 128],
                           rhs=sbk["ident"][:], is_transpose=True,
                           start=True, stop=True)
                    interf = pool.tile([128, 512], BF, tag="interf")
                    nc.vector.tensor_tensor(out=interf[:], in0=pst[:],
                                            in1=bas[:],
                                            op=mybir.AluOpType.add)

                    ph1 = psH.tile([128, 512], FP, tag="ph1")
                    mm(ph1[:], lhsT=sbk["w1pi_bd"][:], rhs=interf[:],
                       start=True, stop=True)
                    h1 = pool.tile([128, 512], BF, tag="h1")
                    if has_bpi1:
                        nc.scalar.activation(h1[:], ph1[:], Tanh,
                                             bias=sbk["b_pi1"][:])
                    else:
                        nc.scalar.activation(h1[:], ph1[:], Tanh)

                    ph2 = psH.tile([128, 512], FP, tag="ph2")
                    mm(ph2[:], lhsT=sbk["wmid_bd"][:], rhs=h1[:],
                       start=True, stop=True)
                    h2 = pool.tile([128, 512], BF, tag="h2")
                    if has_bmid:
                        nc.scalar.activation(h2[:], ph2[:], Tanh,
                                             bias=sbk["b_mid"][:])
                    else:
                        nc.scalar.activation(h2[:], ph2[:], Tanh)

                    pse = psE.tile([128, 512], FP, tag="pse")
                    for kk in range(4):
                        mm(pse[:, 128 * kk:128 * kk + 128],
                           lhsT=h2[:, 128 * kk:128 * kk + 128],
                           rhs=sbk["w2ii_bd"][:], start=True,
                           stop=not has_bii2)
                        if has_bii2:
                            mm(pse[:, 128 * kk:128 * kk + 128],
                               lhsT=sbk["ones_row"][:, :],
                               rhs=sbk["bii2_row"][:, :],
                               start=False, stop=True)
                    iiem = pool.tile([128, 512], BF, tag="iiem")
                    nc.vector.tensor_copy(iiem[:], pse[:])

                    oh = pool.tile([128, TPG * WIN], BF, tag="oh")
                    nc.vector.tensor_tensor(
                        out=oh[:].rearrange("p (b w) -> p b w", b=TPG),
                        in0=lc[:, qq * TPG:(qq + 1) * TPG]
                              .to_broadcast([128, TPG, WIN]),
                        in1=sbk["iota"][:].rearrange("p (b w) -> p b w",
                                                     b=TPG),
                        op=mybir.AluOpType.is_equal)

                    pss = psS.tile([WIN, 512], FP, tag="pss")
                    for t in range(TPG):
                        mm(pss[:, 64 * t:64 * t + 64],
                           lhsT=oh[:, WIN * t:WIN * t + WIN],
                           rhs=iiem[:, 64 * t:64 * t + 64],
                           start=True, stop=True)
                    s_sb = pool.tile([WIN, 512], BF, tag="s_sb")
                    nc.scalar.activation(s_sb[:], pss[:], Copy)
                    nc.sync.dma_start(
                        stage[b][TPG * g:TPG * (g + 1), :, 0:64]
                            .rearrange("t w f -> w t f"),
                        s_sb[:].rearrange("w (t f) -> w t f", t=TPG))

    # ---------------- phase C: merge stages -> out ----------------
    NB1 = C1 // 128
    NB2 = C2 // 128
    if "C" in _PH:
      with TileContext(nc) as tc:
        with tc.tile_pool(name="metaC", bufs=1) as mpool, \
             tc.tile_pool(name="gatC", bufs=1) as gpool, \
             tc.tile_pool(name="sbC", bufs=1) as pool:
            slabs = []
            for b in range(NJB):
                fx = mpool.tile([128, NOUT // 16], I16, tag=f"fx{b}")
                nc.sync.dma_start(fx[:], t_fidx[b])
                sl = gpool.tile([128, NOUT // 128, D], BF, tag=f"sl{b}")
                srows = stage[b][:].rearrange("t w f -> (t w) f")[:, 0:64]
                dma_gather_raw(
                    nc, sl[:, 0:NB1, :], srows, fx[0:16, 0:C1 // 16],
                    num_idxs=C1, elem_size=D, elem_step=128,
                    queue_num=b % 2)
                if C2:
                    dma_gather_raw(
                        nc, sl[:, NB1:NB1 + NB2, :], srows,
                        fx[0:16, C1 // 16:],
                        num_idxs=C2, elem_size=D, elem_step=128,
                        queue_num=b % 2)
                slabs.append(sl)
            acc01 = pool.tile([128, (NOUT // 128) * D], BF, tag="acc01")
            nc.vector.tensor_tensor(
                out=acc01[:].rearrange("p (b f) -> p b f", b=NOUT // 128),
                in0=slabs[0][:], in1=slabs[1][:], op=mybir.AluOpType.add)
            acc23 = pool.tile([128, (NOUT // 128) * D], BF, tag="acc23")
            nc.vector.tensor_tensor(
                out=acc23[:].rearrange("p (b f) -> p b f", b=NOUT // 128),
                in0=slabs[2][:], in1=slabs[3][:], op=mybir.AluOpType.add)
            accf = pool.tile([128, (NOUT // 128) * D], FP, tag="accf")
            nc.vector.tensor_tensor(out=accf[:], in0=acc01[:],
                                    in1=acc23[:], op=mybir.AluOpType.add)
            nc.sync.dma_start(
                t_out[:].rearrange("(p b) f -> p b f", b=NOUT // 128),
                accf[:].rearrange("p (b f) -> p b f", b=NOUT // 128))
    nc.compile()


# ----------------------------------------------------------------- kernel()

SHARED_NAMES = ["w1pp_bd", "w2pp_bd", "w1pi_bd", "wmid_bd", "w2ii_bd",
                "ident", "iota", "b_pp1", "b_pi1", "b_mid", "ones_row",
                "bpp2_row", "bii2_row", "zrow"]
PER_CORE_NAMES = ["p1s", "bas_g", "meta", "loc", "fidx"]


def make_in_maps(per_core, consts):
    shared = {nm: consts[nm] for nm in SHARED_NAMES}
    in_maps = []
    for c in range(NCORES):
        m = dict(shared)
        for nm in PER_CORE_NAMES:
            m[nm] = per_core[c][nm]
        in_maps.append(m)
    return in_maps


def kernel(**inputs):
    idx_i = np.asarray(inputs["idx_i"]).astype(np.int64)
    idx_j = np.asarray(inputs["idx_j"]).astype(np.int64)
    p1 = np.asarray(inputs["p1"], dtype=NPF)
    basis = np.asarray(inputs["basis"], dtype=NPF)
    weights = {k: np.asarray(inputs[k], dtype=NPF) for k in
               ["pp_w1", "pp_b1", "pp_w2", "pp_b2",
                "pi_w1", "pi_b1", "pi_w2", "pi_b2",
                "ii_w1", "ii_b1", "ii_w2", "ii_b2"]}

    per_core, consts, dims = prep(idx_i, idx_j, p1, basis, weights)

    nc = make_nc()
    build(nc, dims, consts)

    res = run_bass_kernel_spmd(nc, make_in_maps(per_core, consts),
                               core_ids=list(range(NCORES)))
    global LAST_EXEC_NS
    LAST_EXEC_NS = res.exec_time_ns

    N = dims["N"]
    nbs = dims["node_bounds"]
    out = np.zeros((N, D), dtype=NPF)
    for c in range(NCORES):
        out[nbs[c]:nbs[c + 1]] = res.results[c]["out"][:nbs[c + 1] - nbs[c]]
    deg = np.bincount(idx_i, minlength=N)
    out[deg == 0] = 0
    return out


# revision 33
# speedup vs baseline: 7.6654x; 1.1051x over previous
"""GCBlock GNN message-passing kernel for 8 Trainium2 NeuronCores.

Strategy (v5 — bulk int16 dma_gather, 4096-row batches):
  * Host: shard edges by destination node range (each core owns a disjoint
    output range -> no collectives). Within a core, sort edges by
    (j-block, i) where j-blocks are 25600-node ranges, so that j-gather
    indices are block-local int16 and i-gather indices are core-local int16.
    Pack edges into 128-edge tiles of whole nodes with node span < 96.
  * Device phase A: compute the pp1 = MLP(p1) node table into DRAM twice:
    an i-table holding this core's node range and four j-block tables,
    bf16 rows padded to 256B (the gather stride field is in 256B units).
    Inputs arrive host-packed in stacked-pair FM layout ([128,512] = two
    64-feature panels on the partition axis) so matmuls/tanh run with
    block-diagonal weights at full 128-partition width.
  * Device phase B (per 4-group batch = 4096 edges): ONE 4096-row
    dma_gather for i-rows + ONE for j-rows (994ns fixed SWDGE cost
    amortized over 4096 descriptors; descriptors read only the 128B
    payload of each 256B row). Per 1024-edge group: DVE add, PE transposes
    to stacked FM, DVE basis add, 3 matmul layers with block-diagonal
    weights, tanh on ScalarE, one-hot scatter matmuls into per-tile
    96-row windows, one psum->sbuf copy and ONE static write of all 8
    windows into a tile-major bf16 stage tensor.
  * Device phase C: per j-block, TWO dma_gathers fetch every output row's
    stage partial; 3 DVE adds + one static write produce the final
    segment sums.
  * All data-dependent structure lives in index tensors; the instruction
    schedule is identical across cores (SPMD single program).
"""

import math
import os

import numpy as np
import ml_dtypes

import concourse.bacc as bacc
import concourse.bass as bass
import concourse.mybir as mybir
from concourse.bass_utils import run_bass_kernel_spmd
from concourse.tile import TileContext

D = 64
TILE = 128            # edges per tile
TPG = 8               # tiles per group
GRP = TILE * TPG      # 1024 edges per group
GB = 4                # groups per gather batch (4096 edges)
BATCH = GRP * GB
MWB = BATCH // 16     # idx columns after 16-partition wrap (256)
NCORES = 8
JB = 25600            # j-block size (int16-safe, multiple of 1024)
NJB = 4
PAD_LOC = 300.0       # one-hot local index for pad edges (matches nothing)
AB = 2                # phase-A steps per load batch

FP = mybir.dt.float32
BF = mybir.dt.bfloat16
I16 = mybir.dt.int16
NPF = np.float32
NPB = ml_dtypes.bfloat16
F8 = mybir.dt.float8e4
NP8 = ml_dtypes.float8_e4m3


def make_nc():
    return bacc.Bacc(trn_type="TRN2", num_swdge_queues=2)


def dma_gather_raw(nc, out_ap, in_ap, idxs_ap, num_idxs, elem_size,
                   elem_step, queue_num=0):
    """dma_gather without the helper's 256B elem minimum / 1024-idx packet.

    The ISA stride field is in 256B units (stride must be %256), but the
    per-descriptor read size is free — reading the 128B payload of padded
    256B rows halves DMA-engine time vs. gathering the full padded row.
    single_packet=False lets num_idxs exceed the 1024-descriptor ring.
    """
    from concourse import ap_utils
    g = nc.gpsimd
    assert idxs_ap.dtype == I16
    assert in_ap.dtype == out_ap.dtype
    stride_bytes = elem_step * mybir.dt.size(in_ap.dtype)
    stride_bytes_256 = stride_bytes // 256
    assert stride_bytes_256 * 256 == stride_bytes and stride_bytes_256 < 256
    assert ap_utils.ap_is_contiguous(out_ap.ap[1:])
    assert ap_utils.ap_is_contiguous(idxs_ap.ap[1:])
    assert in_ap.ap[0][0] == elem_step
    assert in_ap.ap[-1][1] == elem_size
    assert out_ap.ap[-1][1] == elem_size
    _in_ap = g.lower_ap_dma(in_ap, for_custom_bir_dma=True)
    _idxs_ap = g.lower_ap(idxs_ap)
    _out_ap = g.lower_ap(out_ap)
    return g.add_instruction(
        mybir.InstDMAGatherAnt(
            name=g.bass.get_next_instruction_name(),
            ins=[*_in_ap, _idxs_ap, g.lower_val_access(g.to_reg(num_idxs))],
            outs=[_out_ap],
            transpose=False,
            num_idxs=num_idxs,
            elem_size=elem_size,
            stride_bytes_256=stride_bytes_256,
            gen_mode=0,
            single_packet=False,
            queue_num=queue_num,
            sbuf_tokens_per_rank=0,
            sbuf_free_dim_per_rank=0,
            sbuf_free_dim_pad_per_rank=0,
            sbuf_byte_offset=0,
        ))


def _wrap16(lin):
    """[n] int16 linear index list -> [128, n//16] SWDGE-wrapped+replicated."""
    n = lin.shape[0]
    w = lin.reshape(n // 16, 16).T
    return np.tile(w, (8, 1)).copy()


def _bd(w):
    """64x64 -> 128x128 block-diagonal (stacked-pair weights)."""
    out = np.zeros((128, 128), dtype=w.dtype)
    out[:64, :64] = w
    out[64:, 64:] = w
    return out


# ---------------------------------------------------------------- host prep

def prep(idx_i, idx_j, p1, basis, weights):
    N, E = p1.shape[0], idx_i.shape[0]
    assert N <= NJB * JB

    order = np.argsort(idx_i, kind="stable")
    si_all = idx_i[order]
    sj_all = idx_j[order]
    sb_all = basis[order]

    # core boundaries snapped to node edges, balancing edge counts
    node_bounds = [0]
    edge_bounds = [0]
    for c in range(1, NCORES):
        pos = min(int(round(c * E / NCORES)), E - 1)
        node_c = max(int(si_all[pos]), node_bounds[-1] + 1)
        node_bounds.append(node_c)
        edge_bounds.append(int(np.searchsorted(si_all, node_c)))
    node_bounds.append(N)
    edge_bounds.append(E)
    NSLM = max(node_bounds[c + 1] - node_bounds[c] for c in range(NCORES))
    NBLK = math.ceil(NSLM / 128)

    # ---- per-core edge organization ----
    WIN = 64
    core_data = []
    for c in range(NCORES):
        s, e = edge_bounds[c], edge_bounds[c + 1]
        nb = node_bounds[c]
        si = si_all[s:e]
        sj = sj_all[s:e]
        sb = sb_all[s:e]
        jb = sj // JB
        sub = np.lexsort((si, jb))
        si, sj, sb, jb = si[sub], sj[sub], sb[sub], jb[sub]
        jb_starts = [int(np.searchsorted(jb, b)) for b in range(NJB)] + [len(jb)]

        per_jb = []
        for b in range(NJB):
            lo, hi = jb_starts[b], jb_starts[b + 1]
            tiles = []  # (estart, ecount, first_node)
            if hi > lo:
                nodes, counts = np.unique(si[lo:hi], return_counts=True)
                estart = lo + np.concatenate([[0], np.cumsum(counts)[:-1]])
                cur = None
                for k in range(len(nodes)):
                    d = int(counts[k])
                    assert d <= TILE
                    n0 = int(nodes[k])
                    if (cur is None or cur[1] + d > TILE
                            or n0 - cur[2] >= WIN):
                        if cur is not None:
                            tiles.append(tuple(cur))
                        cur = [int(estart[k]), 0, n0]
                    cur[1] += d
                if cur is not None:
                    tiles.append(tuple(cur))
            per_jb.append(tiles)
        core_data.append(dict(nb=nb, si=si, sj=sj, sb=sb, per_jb=per_jb))

    NTJB = max(len(cd["per_jb"][b]) for cd in core_data for b in range(NJB))
    NGJB = math.ceil(math.ceil(NTJB / TPG) / GB) * GB
    NTJB = NGJB * TPG
    assert WIN * (NTJB + 1) <= 32767, (WIN, NTJB)
    NG = NGJB * NJB
    NGB = NG // GB  # gather batches (all groups of a batch share one jb)

    NSI = math.ceil(NBLK * 128 / 1024)
    NASG = math.ceil(N / 1024)
    NAS = NSI + NASG
    NAS = math.ceil(NAS / AB) * AB
    NBAT = math.ceil(NBLK * 128 / GRP)
    NOUT = NBAT * GRP
    # phase C gather split per jb: two instructions
    C1 = NOUT
    C2 = 0

    per_core = []
    for c in range(NCORES):
        cd = core_data[c]
        nb, si, sj, sb = cd["nb"], cd["si"], cd["sj"], cd["sb"]

        meta = np.zeros((NGB, 128, 2 * MWB), np.int16)
        loc = np.full((NG, 128, TPG), PAD_LOC, NPF)
        bas_g = np.zeros((NG, 128, 4 * TILE), NPF)

        for b in range(NJB):
            tiles = cd["per_jb"][b]
            for qb in range(NGJB // GB):
                gi_lin = np.zeros((BATCH,), np.int16)
                gj_lin = np.zeros((BATCH,), np.int16)
                for gg in range(GB):
                    g = qb * GB + gg
                    gidx = b * NGJB + g
                    for t in range(TPG):
                        ti = g * TPG + t
                        if ti >= len(tiles):
                            continue
                        es, cnt, fn = tiles[ti]
                        if cnt == 0:
                            continue
                        o = gg * GRP + t * TILE
                        gi_lin[o:o + cnt] = (si[es:es + cnt] - nb
                                             ).astype(np.int16)
                        gj_lin[o:o + cnt] = (sj[es:es + cnt] - JB * b
                                             ).astype(np.int16)
                        loc[gidx, :cnt, t] = (si[es:es + cnt] - fn
                                              ).astype(NPF)
                        kk, h = t // 2, t % 2
                        bas_g[gidx, 64 * h:64 * h + 64,
                              128 * kk:128 * kk + cnt] = sb[es:es + cnt].T
                bidx = b * (NGJB // GB) + qb
                meta[bidx, :, :MWB] = _wrap16(gi_lin)
                meta[bidx, :, MWB:] = _wrap16(gj_lin)

        # phase C: fidx[jb] -> stage row (t*WIN + w) or the zeroed dump row
        fidx = np.zeros((NJB, 128, NOUT // 16), np.int16)
        for b in range(NJB):
            tiles = cd["per_jb"][b]
            node2row = np.full((NOUT,), NTJB * WIN, np.int32)
            for ti, (es, cnt, fn) in enumerate(tiles):
                if cnt == 0:
                    continue
                nn = np.unique(si[es:es + cnt])
                node2row[nn - nb] = ti * WIN + (nn - fn)
            nblocks = NOUT // 128
            r = np.arange(NOUT)
            perm = (r % 128) * nblocks + r // 128
            n2r = node2row[perm].astype(np.int16)
            fidx[b, :, :C1 // 16] = _wrap16(n2r[:C1])
            if C2:
                fidx[b, :, C1 // 16:] = _wrap16(n2r[C1:])

        # phase A input packing (stacked pairs)
        p1s = np.zeros((NAS, 128, 512), NPF)
        rows_pad = np.zeros((NAS * 1024, 64), NPF)
        for st in range(NSI):
            g0 = nb + 1024 * st
            g1 = min(g0 + 1024, N)
            if g1 > g0:
                rows_pad[st * 1024: st * 1024 + (g1 - g0)] = p1[g0:g1]
        for st in range(NASG):
            g0 = 1024 * st
            g1 = min(g0 + 1024, N)
            rows_pad[(NSI + st) * 1024: (NSI + st) * 1024 + (g1 - g0)] = \
                p1[g0:g1]
        r4 = rows_pad.reshape(NAS, 4, 2, 128, 64)  # st, u, s, p, f
        for u in range(4):
            for sg in range(2):
                p1s[:, 64 * sg:64 * sg + 64, 128 * u:128 * u + 128] = \
                    r4[:, u, sg].transpose(0, 2, 1)

        per_core.append(dict(
            p1s=p1s.astype(NPB),
            bas_g=bas_g.astype(NPB),
            meta=meta,
            loc=loc.astype(NPB),
            fidx=fidx,
        ))

    w = weights
    W_mid = (w["pi_w2"] @ w["ii_w1"]).astype(NPF)
    b_mid = (w["pi_b2"] @ w["ii_w1"] + w["ii_b1"]).astype(NPF)

    def stack_b(b):
        return np.concatenate([b, b]).reshape(128, 1).astype(NPF)

    consts = dict(
        w1pp_bd=_bd(w["pp_w1"].astype(NPF)).astype(NPB),
        w2pp_bd=_bd(w["pp_w2"].astype(NPF)).astype(NPB),
        w1pi_bd=_bd(w["pi_w1"].astype(NPF)).astype(NPB),
        wmid_bd=_bd(W_mid).astype(NPB),
        w2ii_bd=_bd(w["ii_w2"].astype(NPF)).astype(NPB),
        ident=np.eye(128, dtype=NPB),
        iota=np.tile(np.arange(WIN, dtype=NPF), (128, TPG)).astype(NPB),
        b_pp1=stack_b(w["pp_b1"]),
        b_pi1=stack_b(w["pi_b1"]),
        b_mid=stack_b(b_mid.reshape(-1)),
        ones_row=np.ones((1, 128), NPB),
        bpp2_row=np.tile(w["pp_b2"], 2).reshape(1, 2 * D).astype(NPB),
        bii2_row=np.tile(w["ii_b2"], 2).reshape(1, 2 * D).astype(NPB),
        zrow=np.zeros((1, D), NPB),
    )

    dims = dict(N=N, E=E, NTJB=NTJB, NGJB=NGJB, NG=NG, NGB=NGB, WIN=WIN,
                NSI=NSI, NAS=NAS, NASG=NASG, NBLK=NBLK, NBAT=NBAT,
                NOUT=NOUT, C1=C1, C2=C2, node_bounds=node_bounds)
    return per_core, consts, dims


# ------------------------------------------------------------- device build

def build(nc, dims, consts):
    NTJB, NGJB, NG, NGB = dims["NTJB"], dims["NGJB"], dims["NG"], dims["NGB"]
    WIN, NSI, NAS = dims["WIN"], dims["NSI"], dims["NAS"]
    NOUT, C1, C2 = dims["NOUT"], dims["C1"], dims["C2"]
    has_bpp1 = bool(np.any(consts["b_pp1"] != 0))
    has_bpp2 = bool(np.any(consts["bpp2_row"].astype(NPF) != 0))
    has_bpi1 = bool(np.any(consts["b_pi1"] != 0))
    has_bmid = bool(np.any(consts["b_mid"] != 0))
    has_bii2 = bool(np.any(consts["bii2_row"].astype(NPF) != 0))

    t_p1s = nc.dram_tensor("p1s", (NAS, 128, 512), BF, kind="ExternalInput")
    t_bas = nc.dram_tensor("bas_g", (NG, 128, 512), BF, kind="ExternalInput")
    t_meta = nc.dram_tensor("meta", (NGB, 128, 2 * MWB), I16,
                            kind="ExternalInput")
    t_loc = nc.dram_tensor("loc", (NG, 128, TPG), BF, kind="ExternalInput")
    t_fidx = nc.dram_tensor("fidx", (NJB, 128, NOUT // 16), I16,
                            kind="ExternalInput")
    cts = {}
    cdt = dict(b_pp1=FP, b_pi1=FP, b_mid=FP)
    for nm in ["w1pp_bd", "w2pp_bd", "w1pi_bd", "wmid_bd", "w2ii_bd",
               "ident", "iota", "b_pp1", "b_pi1", "b_mid", "ones_row",
               "bpp2_row", "bii2_row", "zrow"]:
        cts[nm] = nc.dram_tensor(nm, consts[nm].shape, cdt.get(nm, BF),
                                 kind="ExternalInput")
    t_out = nc.dram_tensor("out", (NOUT, D), FP, kind="ExternalOutput")

    jtab = [nc.dram_tensor(f"jtab{b}", (JB, 128), BF, kind="Internal")
            for b in range(NJB)]
    itab = nc.dram_tensor("itab", (NSI * 1024, 128), BF, kind="Internal")
    stage = [nc.dram_tensor(f"stage{b}", (NTJB + 1, WIN, 128), BF,
                            kind="Internal")
             for b in range(NJB)]

    def load_consts(pool):
        sb = {}
        for nm, t in cts.items():
            tile = pool.tile(list(consts[nm].shape), cdt.get(nm, BF), tag=nm)
            nc.sync.dma_start(tile[:], t[:])
            sb[nm] = tile
        return sb

    Tanh = mybir.ActivationFunctionType.Tanh
    Copy = mybir.ActivationFunctionType.Copy

    def mm(out, lhsT, rhs, **kw):
        nc.tensor.matmul(out, lhsT=lhsT, rhs=rhs, **kw)

    _PH = os.environ.get("GC_PHASES", "ABC")

    # ---------------- phase A: pp1 tables ----------------
    if "A" in _PH:
      with TileContext(nc) as tc:
        with tc.tile_pool(name="cstA", bufs=1) as cpool, \
             tc.tile_pool(name="ldA", bufs=2) as lpool, \
             tc.tile_pool(name="sbA", bufs=3) as pool, \
             tc.tile_pool(name="psA", bufs=2, space="PSUM") as psA, \
             tc.tile_pool(name="psA2", bufs=2, space="PSUM") as psA2:
            sbk = load_consts(cpool)
            STA2 = math.ceil(NSI / AB) * AB  # only itab steps stay here
            for sb0 in range(0, STA2, AB):
                p1c = lpool.tile([128, AB * 512], BF, tag="p1c")
                nc.sync.dma_start(
                    p1c[:].rearrange("p (a c) -> p a c", a=AB),
                    t_p1s[sb0:sb0 + AB].rearrange("a p c -> p a c"))
                for a in range(AB):
                    st = sb0 + a
                    ps1 = psA.tile([128, 512], FP, tag="ps1")
                    mm(ps1[:], lhsT=sbk["w1pp_bd"][:],
                       rhs=p1c[:, 512 * a:512 * a + 512],
                       start=True, stop=True)
                    h1 = pool.tile([128, 512], BF, tag="h1a")
                    if has_bpp1:
                        nc.scalar.activation(h1[:], ps1[:], Tanh,
                                             bias=sbk["b_pp1"][:])
                    else:
                        nc.scalar.activation(h1[:], ps1[:], Tanh)
                    ps2 = psA2.tile([128, 512], FP, tag="ps2")
                    for u in range(4):
                        mm(ps2[:, 128 * u:128 * u + 128],
                           lhsT=h1[:, 128 * u:128 * u + 128],
                           rhs=sbk["w2pp_bd"][:], start=True,
                           stop=not has_bpp2)
                        if has_bpp2:
                            mm(ps2[:, 128 * u:128 * u + 128],
                               lhsT=sbk["ones_row"][:, :],
                               rhs=sbk["bpp2_row"][:, :],
                               start=False, stop=True)
                    tsb = pool.tile([128, 512], BF, tag="tsb")
                    nc.vector.tensor_copy(tsb[:], ps2[:])
                    if st < NSI:
                        dst = itab[1024 * st:1024 * (st + 1), 0:64]
                    else:
                        g0 = (st - NSI) * 1024
                        b = min(g0 // JB, NJB - 1)
                        r0 = g0 - b * JB
                        dst = jtab[b][r0:r0 + 1024, 0:64]
                    nc.sync.dma_start(
                        dst.rearrange("(b p) f -> p b f", p=128),
                        tsb[:].rearrange("p (b f) -> p b f", b=8))

    # ---------------- phase B: edges ----------------
    if "B" in _PH:
      with TileContext(nc) as tc:
        with tc.tile_pool(name="cstB", bufs=1) as cpool, \
             tc.tile_pool(name="meta", bufs=2) as mpool, \
             tc.tile_pool(name="gat", bufs=2) as gpool, \
             tc.tile_pool(name="sbB", bufs=3) as pool, \
             tc.tile_pool(name="psT", bufs=2, space="PSUM") as psT, \
             tc.tile_pool(name="psH", bufs=1, space="PSUM") as psH, \
             tc.tile_pool(name="psE", bufs=2, space="PSUM") as psE, \
             tc.tile_pool(name="psS", bufs=2, space="PSUM") as psS:
            sbk = load_consts(cpool)
            # tail of phase A (jtab3) — overlaps with jb0-2 batches below
            STA2 = math.ceil(NSI / AB) * AB
            if "A" in _PH:
                for sb0 in range(STA2, NAS, AB):
                    p1c = mpool.tile([128, AB * 512], BF, tag="p1c")
                    nc.sync.dma_start(
                        p1c[:].rearrange("p (a c) -> p a c", a=AB),
                        t_p1s[sb0:sb0 + AB].rearrange("a p c -> p a c"))
                    for a in range(AB):
                        st = sb0 + a
                        ps1 = psH.tile([128, 512], FP, tag="ph1")
                        mm(ps1[:], lhsT=sbk["w1pp_bd"][:],
                           rhs=p1c[:, 512 * a:512 * a + 512],
                           start=True, stop=True)
                        h1 = pool.tile([128, 512], BF, tag="h1a")
                        if has_bpp1:
                            nc.scalar.activation(h1[:], ps1[:], Tanh,
                                                 bias=sbk["b_pp1"][:])
                        else:
                            nc.scalar.activation(h1[:], ps1[:], Tanh)
                        ps2 = psE.tile([128, 512], FP, tag="pse")
                        for u in range(4):
                            mm(ps2[:, 128 * u:128 * u + 128],
                               lhsT=h1[:, 128 * u:128 * u + 128],
                               rhs=sbk["w2pp_bd"][:], start=True,
                               stop=not has_bpp2)
                            if has_bpp2:
                                mm(ps2[:, 128 * u:128 * u + 128],
                                   lhsT=sbk["ones_row"][:, :],
                                   rhs=sbk["bpp2_row"][:, :],
                                   start=False, stop=True)
                        tsb = pool.tile([128, 512], BF, tag="tsb")
                        nc.vector.tensor_copy(tsb[:], ps2[:])
                        g0 = (st - NSI) * 1024
                        b3 = min(g0 // JB, NJB - 1)
                        r0 = g0 - b3 * JB
                        dst = jtab[b3][r0:r0 + 1024, 0:64]
                        nc.sync.dma_start(
                            dst.rearrange("(b p) f -> p b f", p=128),
                            tsb[:].rearrange("p (b f) -> p b f", b=8))
            # zero the dedicated dump row of every stage tensor
            for b in range(NJB):
                srows = stage[b][:].rearrange("t w f -> (t w) f")
                nc.sync.dma_start(
                    srows[NTJB * WIN:NTJB * WIN + 1, 0:64], sbk["zrow"][:])
            for bidx in range(NGB):
                q0 = bidx * GB
                b = q0 // NGJB
                mt = mpool.tile([128, 2 * MWB], I16, tag="mt")
                nc.sync.dma_start(mt[:], t_meta[bidx])
                lc = mpool.tile([128, GB * TPG], BF, tag="lc")
                nc.sync.dma_start(
                    lc[:].rearrange("p (q c) -> p q c", q=GB),
                    t_loc[q0:q0 + GB].rearrange("q p c -> p q c"))
                bas4 = mpool.tile([128, GB * 512], BF, tag="bas4")
                nc.sync.dma_start(
                    bas4[:].rearrange("p (q c) -> p q c", q=GB),
                    t_bas[q0:q0 + GB].rearrange("q p c -> p q c"))

                gi = gpool.tile([128, GB * TPG, 64], BF, tag="gi")
                dma_gather_raw(
                    nc, gi[:], itab[:, 0:64], mt[0:16, 0:MWB],
                    num_idxs=BATCH, elem_size=64, elem_step=128,
                    queue_num=0)
                gj = gpool.tile([128, GB * TPG, 64], BF, tag="gj")
                dma_gather_raw(
                    nc, gj[:], jtab[b][:, 0:64], mt[0:16, MWB:2 * MWB],
                    num_idxs=BATCH, elem_size=64, elem_step=128,
                    queue_num=1)

                for qq in range(GB):
                    gidx = q0 + qq
                    g = gidx - b * NGJB
                    bas = bas4[:, qq * 512:qq * 512 + 512]

                    gsum = pool.tile([128, 512], BF, tag="gsum")
                    nc.vector.tensor_tensor(
                        out=gsum[:].rearrange("p (b f) -> p b f", b=TPG),
                        in0=gi[:, qq * TPG:(qq + 1) * TPG, :],
                        in1=gj[:, qq * TPG:(qq + 1) * TPG, :],
                        op=mybir.AluOpType.add)

                    pst = psT.tile([128, 512], BF, tag="pst")
                    for kk in range(4):
                        mm(pst[:, 128 * kk:128 * kk + 128],
                           lhsT=gsum[:, 128 * kk:128 * kk + 128],
                           rhs=sbk["ident"][:], is_transpose=True,
                           start=True, stop=True)
                    interf = pool.tile([128, 512], BF, tag="interf")
                    nc.vector.tensor_tensor(out=interf[:], in0=pst[:],
                                            in1=bas[:],
                                            op=mybir.AluOpType.add)

                    ph1 = psH.tile([128, 512], FP, tag="ph1")
                    mm(ph1[:], lhsT=sbk["w1pi_bd"][:], rhs=interf[:],
                       start=True, stop=True)
                    h1 = pool.tile([128, 512], BF, tag="h1")
                    if has_bpi1:
                        nc.scalar.activation(h1[:], ph1[:], Tanh,
                                             bias=sbk["b_pi1"][:])
                    else:
                        nc.scalar.activation(h1[:], ph1[:], Tanh)

                    ph2 = psH.tile([128, 512], FP, tag="ph2")
                    mm(ph2[:], lhsT=sbk["wmid_bd"][:], rhs=h1[:],
                       start=True, stop=True)
                    h2 = pool.tile([128, 512], BF, tag="h2")
                    if has_bmid:
                        nc.scalar.activation(h2[:], ph2[:], Tanh,
                                             bias=sbk["b_mid"][:])
                    else:
                        nc.scalar.activation(h2[:], ph2[:], Tanh)

                    pse = psE.tile([128, 512], FP, tag="pse")
                    for kk in range(4):
                        mm(pse[:, 128 * kk:128 * kk + 128],
                           lhsT=h2[:, 128 * kk:128 * kk + 128],
                           rhs=sbk["w2ii_bd"][:], start=True,
                           stop=not has_bii2)
                        if has_bii2:
                            mm(pse[:, 128 * kk:128 * kk + 128],
                               lhsT=sbk["ones_row"][:, :],
                               rhs=sbk["bii2_row"][:, :],
                               start=False, stop=True)
                    iiem = pool.tile([128, 512], BF, tag="iiem")
                    nc.vector.tensor_copy(iiem[:], pse[:])

                    oh = pool.tile([128, TPG * WIN], BF, tag="oh")
                    nc.vector.tensor_tensor(
                        out=oh[:].rearrange("p (b w) -> p b w", b=TPG),
                        in0=lc[:, qq * TPG:(qq + 1) * TPG]
                              .to_broadcast([128, TPG, WIN]),
                        in1=sbk["iota"][:].rearrange("p (b w) -> p b w",
                                                     b=TPG),
                        op=mybir.AluOpType.is_equal)

                    pss = psS.tile([WIN, 512], FP, tag="pss")
                    for t in range(TPG):
                        mm(pss[:, 64 * t:64 * t + 64],
                           lhsT=oh[:, WIN * t:WIN * t + WIN],
                           rhs=iiem[:, 64 * t:64 * t + 64],
                           start=True, stop=True)
                    s_sb = pool.tile([WIN, 512], BF, tag="s_sb")
                    nc.scalar.activation(s_sb[:], pss[:], Copy)
                    nc.sync.dma_start(
                        stage[b][TPG * g:TPG * (g + 1), :, 0:64]
                            .rearrange("t w f -> w t f"),
                        s_sb[:].rearrange("w (t f) -> w t f", t=TPG))

            if "C" in _PH:
                NH = NOUT // 2
                NBH = NH // 128
                NBLOCKS = NOUT // 128
                for half in range(2):
                    i0 = half * NH
                    slabs = []
                    for b in range(NJB):
                        fx = mpool.tile([128, NH // 16], I16,
                                        tag=f"fx{b}{half}")
                        nc.sync.dma_start(
                            fx[:], t_fidx[b][:, i0 // 16:(i0 + NH) // 16])
                        sl = gpool.tile([128, NBH, D], BF, tag=f"sl{b}")
                        srows = stage[b][:] \
                            .rearrange("t w f -> (t w) f")[:, 0:64]
                        dma_gather_raw(
                            nc, sl[:], srows, fx[0:16, :],
                            num_idxs=NH, elem_size=D, elem_step=128,
                            queue_num=b % 2)
                        slabs.append(sl)
                    acc01 = pool.tile([128, NBH * D], BF, tag="acc01")
                    nc.vector.tensor_tensor(
                        out=acc01[:].rearrange("p (b f) -> p b f", b=NBH),
                        in0=slabs[0][:], in1=slabs[1][:],
                        op=mybir.AluOpType.add)
                    acc23 = pool.tile([128, NBH * D], BF, tag="acc23")
                    nc.vector.tensor_tensor(
                        out=acc23[:].rearrange("p (b f) -> p b f", b=NBH),
                        in0=slabs[2][:], in1=slabs[3][:],
                        op=mybir.AluOpType.add)
                    accf = pool.tile([128, NBH * D], FP, tag="accf")
                    nc.vector.tensor_tensor(out=accf[:], in0=acc01[:],
                                            in1=acc23[:],
                                            op=mybir.AluOpType.add)
                    nc.sync.dma_start(
                        t_out[:].rearrange("(p b) f -> p b f", b=NBLOCKS)
                            [:, half * NBH:(half + 1) * NBH, :],
                        accf[:].rearrange("p (b f) -> p b f", b=NBH))

    # ---------------- phase C: merge stages -> out ----------------
    # (emitted at the tail of phase B's context so slab gathers can begin
    #  as soon as each j-block's stage writes retire)
    nc.compile()


# ----------------------------------------------------------------- kernel()

SHARED_NAMES = ["w1pp_bd", "w2pp_bd", "w1pi_bd", "wmid_bd", "w2ii_bd",
                "ident", "iota", "b_pp1", "b_pi1", "b_mid", "ones_row",
                "bpp2_row", "bii2_row", "zrow"]
PER_CORE_NAMES = ["p1s", "bas_g", "meta", "loc", "fidx"]


def make_in_maps(per_core, consts):
    shared = {nm: consts[nm] for nm in SHARED_NAMES}
    in_maps = []
    for c in range(NCORES):
        m = dict(shared)
        for nm in PER_CORE_NAMES:
            m[nm] = per_core[c][nm]
        in_maps.append(m)
    return in_maps


def kernel(**inputs):
    idx_i = np.asarray(inputs["idx_i"]).astype(np.int64)
    idx_j = np.asarray(inputs["idx_j"]).astype(np.int64)
    p1 = np.asarray(inputs["p1"], dtype=NPF)
    basis = np.asarray(inputs["basis"], dtype=NPF)
    weights = {k: np.asarray(inputs[k], dtype=NPF) for k in
               ["pp_w1", "pp_b1", "pp_w2", "pp_b2",
                "pi_w1", "pi_b1", "pi_w2", "pi_b2",
                "ii_w1", "ii_b1", "ii_w2", "ii_b2"]}

    per_core, consts, dims = prep(idx_i, idx_j, p1, basis, weights)

    nc = make_nc()
    build(nc, dims, consts)

    res = run_bass_kernel_spmd(nc, make_in_maps(per_core, consts),
                               core_ids=list(range(NCORES)))
    global LAST_EXEC_NS
    LAST_EXEC_NS = res.exec_time_ns

    N = dims["N"]
    nbs = dims["node_bounds"]
    out = np.zeros((N, D), dtype=NPF)
    for c in range(NCORES):
        out[nbs[c]:nbs[c + 1]] = res.results[c]["out"][:nbs[c + 1] - nbs[c]]
    deg = np.bincount(idx_i, minlength=N)
    out[deg == 0] = 0
    return out
